# revision 47
# baseline (speedup 1.0000x reference)
"""Trainium2 Bass kernel for nn_AttentionSumReader (segment_reduce).

Pipeline per batch (B=64, S=4096, E=128, 600 entities -> logits over first 512):
  scores = doc_emb @ query          (per-batch matvec)
  attn   = masked softmax(scores)   (mask: s < max(seq_length,1))
  sums   = segment_sum(attn, doc_ids)[:512]
  out    = log(sums + 1e-9)

Strategy (v3 — JIT length-specialized flat tile stream + fast-path tail):
  - Data-parallel over batch: 8 batches/core, LOAD-BALANCED across cores by
    valid length (seq_length known on host pre-compile); only the valid
    prefix of each batch is streamed. Host pre-transposes doc to [E, s]
    f16 and packs per-core tile streams (tile = 128 positions). Programs
    are compiled per realized (NT, F) and cached. ~2x traffic from
    f32->f16, ~1.7x from skipping invalid positions, no on-chip transpose.
  - Per-tile batch context is data, not control flow: host sends per-tile
    query columns, additive masks, id hi/lo (int16), so one SPMD program
    serves per-core variable batch boundaries.
  - Matvec: docT tile stationary, per-tile q column moving; scores [128,ch]
    in PSUM. attn = exp(s) via exp(s/4)^4 (ACT), -2000 additive mask;
    normalization deferred (logits need u/Z once at the end).
  - Segment-sum: id = hi*32+lo (hi<19, lo<32). One-hots on DVE in 2-byte
    2x mode (l-major layout, int16 ids vs int16 iota -> bf16). lo-one-hot
    col 32 is constant 1 so each u block's col 32 accumulates per-hi attn
    sums (gives Z). Per-tile matmul (lhsT=w_hi*attn [128,19],
    rhs=oh_lo [128,33]) accumulates per-GROUP u[19,33] in PSUM (group=2
    tiles); batches own whole groups.
  - Group->batch reduction, two paths into per-batch PSUM A[8,429]/B[8,198]
    (= [8, hi, 33] split at hi 13):
    * banked groups (all but last F): u banks [19, 15*33] -> bf16 (ACT) ->
      DRAM -> reload with groups on partitions -> Sel[g, slot] matmul
      (Sel host-built per core). Round-trips overlap the stream (SWDGE
      queue, keeps HWDGE free for the doc stream).
    * fast groups (last F, host guarantees they belong to slot 7 = the
      longest batch, or are zero-mass dummies): u[19,33] -> bf16 -> 19
      identity-column matmuls add rows straight into A/B[7] - no DRAM
      round-trip on the critical tail.
  - Finalize: Z_j = sum_hi A/B[j, hi*33+32]; logits = Ln(u*invZ + eps).
"""

import sys

sys.path.insert(0, "/opt/trn_rl_repo")

from contextlib import ExitStack

import numpy as np
import ml_dtypes

from concourse import bacc, bass, mybir, tile
from concourse import bass_utils

BF16NP = ml_dtypes.bfloat16

# ---- problem constants (hardcoded; kernel.py must be self-contained) ----
B, S, E = 64, 4096, 128
NCORES = 8
BL = B // NCORES  # batches per core
LO = 33  # 32 lo values + 1 ones-column (for Z)
HI = 19  # 600 entities <= 19*32
GPB = 15  # groups per PSUM bank: 15*33*4B = 1980 <= 2048
CHMAX = 16  # max tiles per processing chunk
FMAX = 64  # max fast-path (no-roundtrip) tail groups
OUTE = 512
EPS = 1e-9

F32 = mybir.dt.float32
F16 = mybir.dt.float16
BF16 = mybir.dt.bfloat16
I16 = mybir.dt.int16

ALU = mybir.AluOpType
AF = mybir.ActivationFunctionType
AX = mybir.AxisListType


def make_plan(seq_length):
    """Balance batches across cores by padded valid-tile count; derive the
    uniform per-core stream length NT, fast-group count F, chunk split."""
    L = np.maximum(np.asarray(seq_length, dtype=np.int64), 1)
    tiles = (L + 127) // 128
    gt = 2 * ((tiles + 1) // 2)  # pad each batch to whole groups (G=2)
    order = np.argsort(-gt, kind="stable")
    loads = [0] * NCORES
    counts = [0] * NCORES
    assign = [[] for _ in range(NCORES)]
    for b in order:
        c = min(
            (i for i in range(NCORES) if counts[i] < BL), key=lambda i: loads[i]
        )
        loads[c] += int(gt[b])
        counts[c] += 1
        assign[c].append(int(b))
    # slot order ascending by length: slot BL-1 (stream-last) = longest batch
    assign = [list(reversed(a)) for a in assign]
    NT = int(max(loads))
    NT = max(NT, 4)
    if NT % 2:
        NT += 1
    Gn = NT // 2
    # fast groups must contain only slot-7 tiles or dummies on EVERY core
    start_g7 = []
    for c in range(NCORES):
        pre = sum(int(gt[b]) for b in assign[c][: BL - 1])
        start_g7.append(pre // 2)
    F = max(0, min(FMAX, Gn - max(start_g7)))
    return {"assign": assign, "gt": gt, "L": L, "NT": NT, "F": F}


def make_chunk_sizes(NT, Gb):
    """Even-sized chunks covering [0, 2*Gb) then [2*Gb, NT): a chunk boundary
    lands exactly on the banked/fast split so the last bank's DRAM round-trip
    starts as early as possible. Small first chunk for a fast pipeline head."""

    def split(n, first=None):
        sizes = []
        if first and n >= first:
            sizes.append(first)
            n -= first
        while n > CHMAX:
            sizes.append(CHMAX)
            n -= CHMAX
        if n:
            sizes.append(n)
        return sizes

    return split(2 * Gb, first=4), split(NT - 2 * Gb)


def emit_kernel(ctx, tc, NT, F, tensors):
    nc = tc.nc
    Gn = NT // 2
    Gb = Gn - F  # banked groups
    NBb = (Gb + GPB - 1) // GPB
    sizes_a, sizes_b = make_chunk_sizes(NT, Gb)
    sizes = sizes_a + sizes_b

    (out, docp, ids_d, selb_d, eye_d, xdram) = tensors

    sb = ctx.enter_context(tc.tile_pool(name="sb", bufs=1))
    docpool = ctx.enter_context(tc.tile_pool(name="docp", bufs=6))
    wkpool = ctx.enter_context(tc.tile_pool(name="wk", bufs=3))
    ohpool = ctx.enter_context(tc.tile_pool(name="oh", bufs=3))
    usbpool = ctx.enter_context(tc.tile_pool(name="usb", bufs=2))
    ufpool = ctx.enter_context(tc.tile_pool(name="ufp", bufs=1))
    xppool = ctx.enter_context(tc.tile_pool(name="xp", bufs=max(NBb, 1)))
    scpool = ctx.enter_context(tc.tile_pool(name="sc", bufs=3, space="PSUM"))
    upool = ctx.enter_context(tc.tile_pool(name="up", bufs=3, space="PSUM"))
    abpool = ctx.enter_context(tc.tile_pool(name="ab", bufs=1, space="PSUM"))

    # ---- small inputs ----
    iota33_t = sb.tile([128, LO * CHMAX], I16)
    nc.gpsimd.iota(
        iota33_t[:], pattern=[[1, LO], [0, CHMAX]], base=0, channel_multiplier=0
    )
    iota19_t = sb.tile([128, HI * CHMAX], I16)
    nc.gpsimd.iota(
        iota19_t[:], pattern=[[1, HI], [0, CHMAX]], base=0, channel_multiplier=0
    )
    iota33 = iota33_t[:]
    iota19 = iota19_t[:]
    # ACT/HWDGE queue: qcols first (gates chunk-0 matvec), then ids
    qcols_sb = sb.tile([128, NT], F16)
    nc.scalar.dma_start(out=qcols_sb[:], in_=docp[:, 0:NT])
    ids_t = sb.tile([128, 2 * NT], I16)
    nc.scalar.dma_start(out=ids_t[:], in_=ids_d)
    idlo = ids_t[:, 0:NT]
    idhi = ids_t[:, NT : 2 * NT]
    selb = sb.tile([GPB, max(NBb, 1) * BL], BF16)
    if NBb:
        nc.gpsimd.dma_start(out=selb[:], in_=selb_d)
    eye19 = sb.tile([HI, HI], BF16)
    nc.gpsimd.dma_start(out=eye19[:], in_=eye_d)
    zcol = sb.tile([128, 1], F32)
    nc.vector.memset(zcol[:], 0.0)
    epscol = sb.tile([BL, 1], F32)
    nc.vector.memset(epscol[:], EPS)
    ones19 = sb.tile([HI, 1], BF16)
    nc.vector.memset(ones19[:], 1.0)
    zrow8 = sb.tile([1, BL], BF16)
    nc.vector.memset(zrow8[:], 0.0)
    zwide = sb.tile([1, 13 * LO], BF16)
    nc.vector.memset(zwide[:], 0.0)
    # pin the combined exp+ln activation table once, up front: the auto
    # table-load pass then inserts nothing (no 1.3us reload before the final
    # Ln on the critical tail)
    from concourse.hw_specs import get_activation_tables

    tabs = list(get_activation_tables(nc.m.arch).items())
    combined = next(
        i for i, (k, v) in enumerate(tabs) if AF.Exp in v and AF.Ln in v
    )
    nc.scalar.add_instruction(
        mybir.InstLoadActFuncSet(
            name=nc.get_next_instruction_name(),
            act_func_set_id=combined,
            ins=[],
            outs=[],
        )
    )

    u_tiles = {}
    A_ps = abpool.tile([BL, 13 * LO], F32, tag="A")
    B_ps = abpool.tile([BL, 6 * LO + 1], F32, tag="B")  # col 198 = Z
    # A/B accumulation bookkeeping: first writer starts, closers stop.
    a_seen = [0]
    b_seen = [0]

    def finalize_bank_copy(b):
        gl = min(GPB, Gb - b * GPB)
        usb_t = usbpool.tile([HI, GPB * LO], BF16, tag="usb")
        nc.vector.tensor_copy(out=usb_t[:, : gl * LO], in_=u_tiles[b][:, : gl * LO])
        return usb_t

    def finalize_bank_rt(b, usb_t):
        gl = min(GPB, Gb - b * GPB)
        # write on the SP queue (its wait is long satisfied; the doc queue has
        # ~800ns/chunk of issue slack); read alone on the Pool queue, where
        # its ~2.3us wait on the write's completion blocks nothing else
        nc.sync.dma_start(
            out=xdram[b * GPB : b * GPB + gl, :].rearrange("g (h l) -> h g l", h=HI),
            in_=usb_t[:, : gl * LO].rearrange("h (g l) -> h g l", g=gl),
        )
        xp_t = xppool.tile([GPB, HI * LO + 1], BF16, tag="xp")
        nc.gpsimd.dma_start(
            out=xp_t[:gl, 0 : HI * LO], in_=xdram[b * GPB : b * GPB + gl, :]
        )

        gl = min(GPB, Gb - b * GPB)

        def mms():
            # per-group total attn (for Z): sum the 19 col-32 entries per row
            with nc.allow_low_precision(reason="Z column; rel err ~2^-8 on ln(Z)"):
                nc.vector.tensor_reduce(
                    out=xp_t[:gl, HI * LO : HI * LO + 1],
                    in_=xp_t[:gl, 0 : HI * LO].rearrange("g (h l) -> g h l", l=LO)[
                        :, :, 32
                    ],
                    axis=AX.X,
                    op=ALU.add,
                )
            a_seen[0] += 1
            nc.tensor.matmul(
                out=A_ps[:],
                lhsT=selb[0:gl, b * BL : (b + 1) * BL],
                rhs=xp_t[0:gl, 0 : 13 * LO],
                start=(a_seen[0] == 1),
                stop=False,
            )
            b_seen[0] += 1
            nc.tensor.matmul(
                out=B_ps[:],
                lhsT=selb[0:gl, b * BL : (b + 1) * BL],
                rhs=xp_t[0:gl, 13 * LO : HI * LO + 1],
                start=(b_seen[0] == 1),
                stop=False,
            )

        return mms

    def finalize_fast():
        # the whole fast region belongs to the stream-last batch, which the
        # host maps to SLOT 0 (PE matmul out base partition must be 0/32/64);
        # it was accumulated into the single uF psum tile. One bf16 copy, then
        # 19 identity-column matmuls scatter its rows into A/B[0], plus one
        # ones-column matmul for its Z contribution.
        uf_sb = ufpool.tile([HI, LO], BF16, tag="uf")
        nc.vector.tensor_copy(out=uf_sb[:], in_=u_tiles["uF"][:])
        a_seen[0] += 1
        b_seen[0] += 1
        for h in range(HI):
            if h < 13:
                o = A_ps[0:1, h * LO : (h + 1) * LO]
                st = a_seen[0] == 1 and h == 0
            else:
                o = B_ps[0:1, (h - 13) * LO : (h - 12) * LO]
                st = b_seen[0] == 1 and h == 13
            nc.tensor.matmul(
                out=o, lhsT=eye19[:, h : h + 1], rhs=uf_sb[:], start=st, stop=False
            )
        nc.tensor.matmul(
            out=B_ps[0:1, 6 * LO : 6 * LO + 1],
            lhsT=ones19[:],
            rhs=uf_sb[:, 32:33],
            start=False,
            stop=False,
        )

    state = {"banks_done": 0}

    def emit_front(t0, ch):
        """doc DMA + matvecs: emitted a chunk ahead so the PE queue never
        head-blocks the next chunk's matvecs behind this chunk's segmats.
"""
        doc_t = docpool.tile([128, CHMAX * 128], F16, tag="doc")
        nc.sync.dma_start(
            out=doc_t[:, : ch * 128],
            in_=docp[:, NT + t0 * 128 : NT + (t0 + ch) * 128],
        )
        scores = scpool.tile([128, CHMAX], F32, tag="sc")
        for tt in range(ch):
            nc.tensor.matmul(
                out=scores[:, tt : tt + 1],
                lhsT=doc_t[:, tt * 128 : (tt + 1) * 128],
                rhs=qcols_sb[:, t0 + tt : t0 + tt + 1],
                start=True,
                stop=True,
            )
        return scores

    def emit_rest(t0, ch, scores):
        # attn = exp(s) straight from PSUM in bf16 (s <= ~83 < ln(bf16 max));
        # masking is free: hosts sets ids=-1 at invalid positions, so both
        # one-hots (incl. the is_ge Z column) are all-zero there
        attn = wkpool.tile([128, CHMAX], BF16, tag="attn")
        nc.scalar.activation(
            out=attn[:, :ch], in_=scores[:, :ch], func=AF.Exp, bias=zcol[:, 0:1], scale=1.0
        )
        oh_t = ohpool.tile([128, LO * CHMAX], BF16, tag="oh")
        oh_v = oh_t[:].rearrange("p (l t) -> p l t", t=CHMAX)
        nc.vector.tensor_tensor(
            out=oh_v[:, :, 0:ch],
            in0=idlo[:, t0 : t0 + ch]
            .rearrange("p (o t) -> p o t", o=1)
            .to_broadcast([128, LO, ch]),
            in1=iota33.rearrange("p (l t) -> p l t", t=CHMAX)[:, :, 0:ch],
            op=ALU.is_equal,
        )
        nc.vector.tensor_scalar(
            out=oh_t[:, 32 * CHMAX : 32 * CHMAX + ch],
            in0=idlo[:, t0 : t0 + ch],
            scalar1=0,
            scalar2=None,
            op0=ALU.is_ge,
        )
        w19 = ohpool.tile([128, HI * CHMAX], BF16, tag="w19")
        w19_v = w19[:].rearrange("p (h t) -> p h t", t=CHMAX)
        nc.vector.tensor_tensor(
            out=w19_v[:, :, 0:ch],
            in0=idhi[:, t0 : t0 + ch]
            .rearrange("p (o t) -> p o t", o=1)
            .to_broadcast([128, HI, ch]),
            in1=iota19.rearrange("p (h t) -> p h t", t=CHMAX)[:, :, 0:ch],
            op=ALU.is_equal,
        )
        w19a = ohpool.tile([128, HI * CHMAX], BF16, tag="w19a")
        w19a_v = w19a[:].rearrange("p (h t) -> p h t", t=CHMAX)
        nc.vector.tensor_tensor(
            out=w19a_v[:, :, 0:ch],
            in0=w19_v[:, :, 0:ch],
            in1=attn[:, :ch]
            .rearrange("p (o t) -> p o t", o=1)
            .to_broadcast([128, HI, ch]),
            op=ALU.mult,
        )
        for tt in range(ch):
            t = t0 + tt
            g = t // 2
            if g < Gb:
                bk = g // GPB
                if bk not in u_tiles:
                    u_tiles[bk] = upool.tile(
                        [HI, GPB * LO], F32, tag="u", name=f"u{bk}"
                    )
                o = u_tiles[bk][:, (g % GPB) * LO : (g % GPB) * LO + LO]
                st = t % 2 == 0
                sp = t % 2 == 1
            else:
                if "uF" not in u_tiles:
                    u_tiles["uF"] = upool.tile([HI, LO], F32, tag="u", name="uF")
                o = u_tiles["uF"][:]
                st = t == 2 * Gb
                sp = t == NT - 1
            nc.tensor.matmul(
                out=o, lhsT=w19a_v[:, :, tt], rhs=oh_v[:, :, tt], start=st, stop=sp
            )
        tend = t0 + ch
        while state["banks_done"] < NBb and 2 * min(
            (state["banks_done"] + 1) * GPB, Gb
        ) <= tend:
            b = state["banks_done"]
            done = 2 * min((b + 1) * GPB, Gb)
            # the LAST bank's chain is the tail's critical path: schedule its
            # copy immediately and its round-trip one step later
            d = 0 if b == NBb - 1 else 16
            pending_copy.append((done + d, b))
            state["banks_done"] += 1
        while pending_copy and pending_copy[0][0] <= tend:
            due, b = pending_copy.pop(0)
            pending_rt.append((due + 16, b, finalize_bank_copy(b)))
        while pending_rt and pending_rt[0][0] <= tend:
            due, b, usb_t = pending_rt.pop(0)
            pending_mms.append((due + 24, finalize_bank_rt(b, usb_t)))
        while pending_mms and pending_mms[0][0] <= tend:
            pending_mms.pop(0)[1]()

    # ---- main stream (software-pipelined emission) ----
    t0 = 0
    prev = None
    pending_copy = []
    pending_rt = []
    pending_mms = []
    for ch in sizes:
        scores = emit_front(t0, ch)
        if prev is not None:
            emit_rest(*prev)
        prev = (t0, ch, scores)
        t0 += ch
    emit_rest(*prev)
    assert state["banks_done"] == NBb, (state["banks_done"], NBb)
    # leftover bank round-trips (their group data is long since ready)
    while pending_copy:
        _, b = pending_copy.pop(0)
        pending_rt.append((0, b, finalize_bank_copy(b)))
    while pending_rt:
        _, b, usb_t = pending_rt.pop(0)
        pending_mms.append((0, finalize_bank_rt(b, usb_t)))
    if F > 0:
        finalize_fast()
    # leftover Sel matmuls go AFTER the fast-path matmuls: their xp read may
    # still be in flight and a parked matmul head-blocks the PE queue
    while pending_mms:
        pending_mms.pop(0)[1]()

    # close both PSUM accumulation groups over their FULL regions (partial
    # sub-region stops don't end the group for the untouched rows)
    nc.tensor.matmul(
        out=A_ps[:], lhsT=zrow8[:], rhs=zwide[:, 0 : 13 * LO], start=False, stop=True
    )
    nc.tensor.matmul(
        out=B_ps[:], lhsT=zrow8[:], rhs=zwide[:, 0 : 6 * LO + 1], start=False, stop=True
    )

    # ---- finalize: invZ, then logits = Ln(u*invZ + eps) straight from PSUM ----
    zz = sb.tile([BL, 1], F32)
    nc.vector.reciprocal(out=zz[:], in_=B_ps[:, 6 * LO : 6 * LO + 1])
    lg = sb.tile([BL, OUTE], F32)
    nc.scalar.activation(
        out=lg[:, 0:416].rearrange("j (h l) -> j h l", h=13),
        in_=A_ps[:].rearrange("j (h l) -> j h l", h=13)[:, :, 0:32],
        func=AF.Ln,
        bias=epscol[:, 0:1],
        scale=zz[:, 0:1],
    )
    nc.scalar.activation(
        out=lg[:, 416:512].rearrange("j (h l) -> j h l", h=3),
        in_=B_ps[:, 0 : 6 * LO].rearrange("j (h l) -> j h l", h=6)[:, 0:3, 0:32],
        func=AF.Ln,
        bias=epscol[:, 0:1],
        scale=zz[:, 0:1],
    )
    nc.sync.dma_start(out=out, in_=lg[:])


def build_program(NT, F):
    Gn = NT // 2
    Gb = Gn - F
    NBb = (Gb + GPB - 1) // GPB
    nc = bacc.Bacc(
        "TRN2",
        target_bir_lowering=False,
        debug=False,
        enable_asserts=False,
        num_devices=1,
    )
    docp = nc.dram_tensor(
        "docp", [128, NT + NT * 128], F16, kind="ExternalInput"
    ).ap()  # cols [0:NT] = per-tile q values; [NT:] = packed docT
    ids_d = nc.dram_tensor("ids", [128, 2 * NT], I16, kind="ExternalInput").ap()
    selb_d = nc.dram_tensor(
        "selb", [GPB, max(NBb, 1) * BL], BF16, kind="ExternalInput"
    ).ap()
    eye_d = nc.dram_tensor("eye19", [HI, HI], BF16, kind="ExternalInput").ap()
    xdram = nc.dram_tensor(
        "xdram", [max(Gb, 1), HI * LO], BF16, kind="ExternalInput"
    ).ap()
    out = nc.dram_tensor("out", [BL, OUTE], F32, kind="ExternalOutput").ap()

    tensors = (out, docp, ids_d, selb_d, eye_d, xdram)
    with tile.TileContext(nc) as tc:
        with ExitStack() as ctx:
            emit_kernel(ctx, tc, NT, F, tensors)
    nc.compile()
    return nc


def make_in_maps(doc_emb, query_emb, doc_ids, seq_length, plan):
    NT = plan["NT"]
    F = plan["F"]
    Gn = NT // 2
    Gb = Gn - F
    NBb = (Gb + GPB - 1) // GPB
    gt = plan["gt"]
    L = plan["L"]
    eye = np.eye(HI, dtype=np.float32).astype(BF16NP)
    in_maps = []
    for c in range(NCORES):
        bs = plan["assign"][c]
        docq = np.zeros((128, NT + NT * 128), np.float16)
        docT = docq[:, NT:].reshape(128, NT, 128)
        qcols = docq[:, :NT]
        ids2 = np.full((128, 2 * NT), -1, np.int16)
        idlo = ids2[:, :NT]
        idhi = ids2[:, NT:]
        selb = np.zeros((GPB, max(NBb, 1) * BL), BF16NP)
        t0 = 0
        p = np.arange(128)
        for j, b in enumerate(bs):
            nt = int(gt[b])
            lj = int(L[b])
            npos = min(nt * 128, S)
            seg = np.zeros((nt * 128, E), np.float32)
            seg[:npos] = doc_emb[b, :npos, :]
            docT[:, t0 : t0 + nt, :] = (
                seg.reshape(nt, 128, E).transpose(2, 0, 1).astype(np.float16)
            )
            qcols[:, t0 : t0 + nt] = query_emb[b].astype(np.float16)[:, None]
            svals = (np.arange(nt) * 128)[None, :] + p[:, None]
            valid = svals < lj
            idseg = np.zeros(nt * 128, np.int32)
            idseg[:npos] = doc_ids[b, :npos]
            idseg = idseg.reshape(nt, 128).T
            idlo[:, t0 : t0 + nt] = np.where(valid, idseg & 31, -1).astype(np.int16)
            idhi[:, t0 : t0 + nt] = np.where(valid, idseg >> 5, -1).astype(np.int16)
            slot = (j + 1) % BL  # stream-last batch -> slot 0 (fast path)
            for g in range(t0 // 2, (t0 + nt) // 2):
                if g < Gb:
                    selb[g % GPB, (g // GPB) * BL + slot] = 1.0
                else:
                    # fast groups are added straight into slot 0
                    assert slot == 0, (c, j, g, Gb)
            t0 += nt
        in_maps.append(
            {
                "docp": docq,
                "ids": ids2,
                "selb": selb,
                "eye19": eye,
                "xdram": np.zeros((max(Gb, 1), HI * LO), BF16NP),
            }
        )
    return in_maps


_CACHE = {}


def _get_program(key=None):
    if key is None:
        key = _CACHE.get("last_key")
        assert key is not None, "no program built yet"
    if key not in _CACHE:
        _CACHE[key] = build_program(*key)
    _CACHE["last_key"] = key
    return _CACHE[key]


def kernel(**inputs):
    doc_emb = np.asarray(inputs["doc_emb"], dtype=np.float32)
    query_emb = np.asarray(inputs["query_emb"], dtype=np.float32)
    doc_ids = np.asarray(inputs["doc_ids"], dtype=np.int32)
    seq_length = np.asarray(inputs["seq_length"], dtype=np.int32)

    plan = make_plan(seq_length)
    nc = _get_program((plan["NT"], plan["F"]))
    in_maps = make_in_maps(doc_emb, query_emb, doc_ids, seq_length, plan)
    res = bass_utils.run_bass_kernel_spmd(nc, in_maps, core_ids=list(range(NCORES)))
    out = np.zeros((B, OUTE), np.float32)
    for c in range(NCORES):
        o = np.asarray(res.results[c]["out"], dtype=np.float32)
        for j, b in enumerate(plan["assign"][c]):
            out[b] = o[(j + 1) % BL]
    return out


# revision 49
# speedup vs baseline: 1.0312x; 1.0312x over previous
"""Trainium2 Bass kernel for nn_AttentionSumReader (segment_reduce).

Pipeline per batch (B=64, S=4096, E=128, 600 entities -> logits over first 512):
  scores = doc_emb @ query          (per-batch matvec)
  attn   = masked softmax(scores)   (mask: s < max(seq_length,1))
  sums   = segment_sum(attn, doc_ids)[:512]
  out    = log(sums + 1e-9)

Strategy (v3 — JIT length-specialized flat tile stream + fast-path tail):
  - Data-parallel over batch: 8 batches/core, LOAD-BALANCED across cores by
    valid length (seq_length known on host pre-compile); only the valid
    prefix of each batch is streamed. Host pre-transposes doc to [E, s]
    f16 and packs per-core tile streams (tile = 128 positions). Programs
    are compiled per realized (NT, F) and cached. ~2x traffic from
    f32->f16, ~1.7x from skipping invalid positions, no on-chip transpose.
  - Per-tile batch context is data, not control flow: host sends per-tile
    query columns, additive masks, id hi/lo (int16), so one SPMD program
    serves per-core variable batch boundaries.
  - Matvec: docT tile stationary, per-tile q column moving; scores [128,ch]
    in PSUM. attn = exp(s) via exp(s/4)^4 (ACT), -2000 additive mask;
    normalization deferred (logits need u/Z once at the end).
  - Segment-sum: id = hi*32+lo (hi<19, lo<32). One-hots on DVE in 2-byte
    2x mode (l-major layout, int16 ids vs int16 iota -> bf16). lo-one-hot
    col 32 is constant 1 so each u block's col 32 accumulates per-hi attn
    sums (gives Z). Per-tile matmul (lhsT=w_hi*attn [128,19],
    rhs=oh_lo [128,33]) accumulates per-GROUP u[19,33] in PSUM (group=2
    tiles); batches own whole groups.
  - Group->batch reduction, two paths into per-batch PSUM A[8,429]/B[8,198]
    (= [8, hi, 33] split at hi 13):
    * banked groups (all but last F): u banks [19, 15*33] -> bf16 (ACT) ->
      DRAM -> reload with groups on partitions -> Sel[g, slot] matmul
      (Sel host-built per core). Round-trips overlap the stream (SWDGE
      queue, keeps HWDGE free for the doc stream).
    * fast groups (last F, host guarantees they belong to slot 7 = the
      longest batch, or are zero-mass dummies): u[19,33] -> bf16 -> 19
      identity-column matmuls add rows straight into A/B[7] - no DRAM
      round-trip on the critical tail.
  - Finalize: Z_j = sum_hi A/B[j, hi*33+32]; logits = Ln(u*invZ + eps).
"""

import sys

sys.path.insert(0, "/opt/trn_rl_repo")

from contextlib import ExitStack

import numpy as np
import ml_dtypes

from concourse import bacc, bass, mybir, tile
from concourse import bass_utils

BF16NP = ml_dtypes.bfloat16

# ---- problem constants (hardcoded; kernel.py must be self-contained) ----
B, S, E = 64, 4096, 128
NCORES = 8
BL = B // NCORES  # batches per core
LO = 33  # 32 lo values + 1 ones-column (for Z)
HI = 19  # 600 entities <= 19*32
GPB = 15  # groups per PSUM bank: 15*33*4B = 1980 <= 2048
CHMAX = 16  # max tiles per processing chunk
FMAX = 64  # max fast-path (no-roundtrip) tail groups
OUTE = 512
EPS = 1e-9

F32 = mybir.dt.float32
F16 = mybir.dt.float16
BF16 = mybir.dt.bfloat16
I16 = mybir.dt.int16

ALU = mybir.AluOpType
AF = mybir.ActivationFunctionType
AX = mybir.AxisListType


def make_plan(seq_length):
    """Balance batches across cores by padded valid-tile count; derive the
    uniform per-core stream length NT, fast-group count F, chunk split."""
    L = np.maximum(np.asarray(seq_length, dtype=np.int64), 1)
    tiles = (L + 127) // 128
    gt = 2 * ((tiles + 1) // 2)  # pad each batch to whole groups (G=2)
    order = np.argsort(-gt, kind="stable")
    loads = [0] * NCORES
    counts = [0] * NCORES
    assign = [[] for _ in range(NCORES)]
    for b in order:
        c = min(
            (i for i in range(NCORES) if counts[i] < BL), key=lambda i: loads[i]
        )
        loads[c] += int(gt[b])
        counts[c] += 1
        assign[c].append(int(b))
    # slot order ascending by length: slot BL-1 (stream-last) = longest batch
    assign = [list(reversed(a)) for a in assign]
    NT = int(max(loads))
    NT = max(NT, 4)
    if NT % 2:
        NT += 1
    Gn = NT // 2
    # fast groups must contain only slot-7 tiles or dummies on EVERY core
    start_g7 = []
    for c in range(NCORES):
        pre = sum(int(gt[b]) for b in assign[c][: BL - 1])
        start_g7.append(pre // 2)
    F = max(0, min(FMAX, Gn - max(start_g7)))
    return {"assign": assign, "gt": gt, "L": L, "NT": NT, "F": F}


def make_chunk_sizes(NT, Gb):
    """Even-sized chunks covering [0, 2*Gb) then [2*Gb, NT): a chunk boundary
    lands exactly on the banked/fast split so the last bank's DRAM round-trip
    starts as early as possible. Small first chunk for a fast pipeline head."""

    def split(n, first=None):
        sizes = []
        if first and n >= first:
            sizes.append(first)
            n -= first
        while n > CHMAX:
            sizes.append(CHMAX)
            n -= CHMAX
        if n:
            sizes.append(n)
        return sizes

    return split(2 * Gb, first=4), split(NT - 2 * Gb)


def emit_kernel(ctx, tc, NT, F, tensors):
    nc = tc.nc
    Gn = NT // 2
    Gb = Gn - F  # banked groups
    NBb = (Gb + GPB - 1) // GPB
    sizes_a, sizes_b = make_chunk_sizes(NT, Gb)
    sizes = sizes_a + sizes_b

    (out, docp, ids_d, selb_d, eye_d, xdram) = tensors

    sb = ctx.enter_context(tc.tile_pool(name="sb", bufs=1))
    docpool = ctx.enter_context(tc.tile_pool(name="docp", bufs=6))
    wkpool = ctx.enter_context(tc.tile_pool(name="wk", bufs=3))
    ohpool = ctx.enter_context(tc.tile_pool(name="oh", bufs=3))
    usbpool = ctx.enter_context(tc.tile_pool(name="usb", bufs=2))
    ufpool = ctx.enter_context(tc.tile_pool(name="ufp", bufs=1))
    xppool = ctx.enter_context(tc.tile_pool(name="xp", bufs=max(NBb, 1)))
    scpool = ctx.enter_context(tc.tile_pool(name="sc", bufs=3, space="PSUM"))
    upool = ctx.enter_context(tc.tile_pool(name="up", bufs=3, space="PSUM"))
    abpool = ctx.enter_context(tc.tile_pool(name="ab", bufs=1, space="PSUM"))

    # ---- small inputs ----
    iota33_t = sb.tile([128, LO * CHMAX], I16)
    nc.gpsimd.iota(
        iota33_t[:], pattern=[[1, LO], [0, CHMAX]], base=0, channel_multiplier=0
    )
    iota19_t = sb.tile([128, HI * CHMAX], I16)
    nc.gpsimd.iota(
        iota19_t[:], pattern=[[1, HI], [0, CHMAX]], base=0, channel_multiplier=0
    )
    iota33 = iota33_t[:]
    iota19 = iota19_t[:]
    # ACT/HWDGE queue: ids in one small DMA (transfers are tiny)
    ids_t = sb.tile([128, 2 * NT], I16)
    nc.scalar.dma_start(out=ids_t[:], in_=ids_d)
    idlo = ids_t[:, 0:NT]
    idhi = ids_t[:, NT : 2 * NT]
    selb = sb.tile([GPB, max(NBb, 1) * BL], BF16)
    if NBb:
        nc.gpsimd.dma_start(out=selb[:], in_=selb_d)
    eye19 = sb.tile([HI, HI], BF16)
    nc.gpsimd.dma_start(out=eye19[:], in_=eye_d)
    zcol = sb.tile([128, 1], F32)
    nc.vector.memset(zcol[:], 0.0)
    epscol = sb.tile([BL, 1], F32)
    nc.vector.memset(epscol[:], EPS)
    ones19 = sb.tile([HI, 1], BF16)
    nc.vector.memset(ones19[:], 1.0)
    zrow8 = sb.tile([1, BL], BF16)
    nc.vector.memset(zrow8[:], 0.0)
    zwide = sb.tile([1, 13 * LO], BF16)
    nc.vector.memset(zwide[:], 0.0)
    # pin the combined exp+ln activation table once, up front: the auto
    # table-load pass then inserts nothing (no 1.3us reload before the final
    # Ln on the critical tail)
    from concourse.hw_specs import get_activation_tables

    tabs = list(get_activation_tables(nc.m.arch).items())
    combined = next(
        i for i, (k, v) in enumerate(tabs) if AF.Exp in v and AF.Ln in v
    )
    nc.scalar.add_instruction(
        mybir.InstLoadActFuncSet(
            name=nc.get_next_instruction_name(),
            act_func_set_id=combined,
            ins=[],
            outs=[],
        )
    )

    u_tiles = {}
    A_ps = abpool.tile([BL, 13 * LO], F32, tag="A")
    B_ps = abpool.tile([BL, 6 * LO + 1], F32, tag="B")  # col 198 = Z
    # A/B accumulation bookkeeping: first writer starts, closers stop.
    a_seen = [0]
    b_seen = [0]

    def finalize_bank_copy(b):
        gl = min(GPB, Gb - b * GPB)
        usb_t = usbpool.tile([HI, GPB * LO], BF16, tag="usb")
        nc.vector.tensor_copy(out=usb_t[:, : gl * LO], in_=u_tiles[b][:, : gl * LO])
        return usb_t

    def finalize_bank_rt(b, usb_t):
        gl = min(GPB, Gb - b * GPB)
        # write on the SP queue (its wait is long satisfied; the doc queue has
        # ~800ns/chunk of issue slack); read alone on the Pool queue, where
        # its ~2.3us wait on the write's completion blocks nothing else
        nc.sync.dma_start(
            out=xdram[b * GPB : b * GPB + gl, :].rearrange("g (h l) -> h g l", h=HI),
            in_=usb_t[:, : gl * LO].rearrange("h (g l) -> h g l", g=gl),
        )
        xp_t = xppool.tile([GPB, HI * LO + 1], BF16, tag="xp")
        nc.gpsimd.dma_start(
            out=xp_t[:gl, 0 : HI * LO], in_=xdram[b * GPB : b * GPB + gl, :]
        )

        gl = min(GPB, Gb - b * GPB)

        def mms():
            # per-group total attn (for Z): sum the 19 col-32 entries per row
            with nc.allow_low_precision(reason="Z column; rel err ~2^-8 on ln(Z)"):
                nc.vector.tensor_reduce(
                    out=xp_t[:gl, HI * LO : HI * LO + 1],
                    in_=xp_t[:gl, 0 : HI * LO].rearrange("g (h l) -> g h l", l=LO)[
                        :, :, 32
                    ],
                    axis=AX.X,
                    op=ALU.add,
                )
            a_seen[0] += 1
            nc.tensor.matmul(
                out=A_ps[:],
                lhsT=selb[0:gl, b * BL : (b + 1) * BL],
                rhs=xp_t[0:gl, 0 : 13 * LO],
                start=(a_seen[0] == 1),
                stop=False,
            )
            b_seen[0] += 1
            nc.tensor.matmul(
                out=B_ps[:],
                lhsT=selb[0:gl, b * BL : (b + 1) * BL],
                rhs=xp_t[0:gl, 13 * LO : HI * LO + 1],
                start=(b_seen[0] == 1),
                stop=False,
            )

        return mms

    def finalize_fast():
        # the whole fast region belongs to the stream-last batch, which the
        # host maps to SLOT 0 (PE matmul out base partition must be 0/32/64);
        # it was accumulated into the single uF psum tile. One bf16 copy, then
        # 19 identity-column matmuls scatter its rows into A/B[0], plus one
        # ones-column matmul for its Z contribution.
        uf_sb = ufpool.tile([HI, LO], BF16, tag="uf")
        nc.vector.tensor_copy(out=uf_sb[:], in_=u_tiles["uF"][:])
        a_seen[0] += 1
        b_seen[0] += 1
        for h in range(HI):
            if h < 13:
                o = A_ps[0:1, h * LO : (h + 1) * LO]
                st = a_seen[0] == 1 and h == 0
            else:
                o = B_ps[0:1, (h - 13) * LO : (h - 12) * LO]
                st = b_seen[0] == 1 and h == 13
            nc.tensor.matmul(
                out=o, lhsT=eye19[:, h : h + 1], rhs=uf_sb[:], start=st, stop=False
            )
        nc.tensor.matmul(
            out=B_ps[0:1, 6 * LO : 6 * LO + 1],
            lhsT=ones19[:],
            rhs=uf_sb[:, 32:33],
            start=False,
            stop=False,
        )

    state = {"banks_done": 0}
    qcols_sb = sb.tile([128, NT], F16)

    def emit_front(t0, ch):
        """doc DMA + matvecs: emitted a chunk ahead so the PE queue never
        head-blocks the next chunk's matvecs behind this chunk's segmats."""
        doc_t = docpool.tile([128, CHMAX * 128], F16, tag="doc")
        nc.sync.dma_start(
            out=doc_t[:, : ch * 128],
            in_=docp[:, NT + t0 * 128 : NT + (t0 + ch) * 128],
        )
        if t0 == 0:
            nc.scalar.dma_start(out=qcols_sb[:], in_=docp[:, 0:NT])
        scores = scpool.tile([128, CHMAX], F32, tag="sc")
        for tt in range(ch):
            nc.tensor.matmul(
                out=scores[:, tt : tt + 1],
                lhsT=doc_t[:, tt * 128 : (tt + 1) * 128],
                rhs=qcols_sb[:, t0 + tt : t0 + tt + 1],
                start=True,
                stop=True,
            )
        return scores

    def emit_rest(t0, ch, scores):
        # attn = exp(s) straight from PSUM in bf16 (s <= ~83 < ln(bf16 max));
        # masking is free: hosts sets ids=-1 at invalid positions, so both
        # one-hots (incl. the is_ge Z column) are all-zero there
        attn = wkpool.tile([128, CHMAX], BF16, tag="attn")
        nc.scalar.activation(
            out=attn[:, :ch], in_=scores[:, :ch], func=AF.Exp, bias=zcol[:, 0:1], scale=1.0
        )
        oh_t = ohpool.tile([128, LO * CHMAX], BF16, tag="oh")
        oh_v = oh_t[:].rearrange("p (l t) -> p l t", t=CHMAX)
        nc.vector.tensor_tensor(
            out=oh_v[:, :, 0:ch],
            in0=idlo[:, t0 : t0 + ch]
            .rearrange("p (o t) -> p o t", o=1)
            .to_broadcast([128, LO, ch]),
            in1=iota33.rearrange("p (l t) -> p l t", t=CHMAX)[:, :, 0:ch],
            op=ALU.is_equal,
        )
        nc.vector.tensor_scalar(
            out=oh_t[:, 32 * CHMAX : 32 * CHMAX + ch],
            in0=idlo[:, t0 : t0 + ch],
            scalar1=0,
            scalar2=None,
            op0=ALU.is_ge,
        )
        w19 = ohpool.tile([128, HI * CHMAX], BF16, tag="w19")
        w19_v = w19[:].rearrange("p (h t) -> p h t", t=CHMAX)
        nc.vector.tensor_tensor(
            out=w19_v[:, :, 0:ch],
            in0=idhi[:, t0 : t0 + ch]
            .rearrange("p (o t) -> p o t", o=1)
            .to_broadcast([128, HI, ch]),
            in1=iota19.rearrange("p (h t) -> p h t", t=CHMAX)[:, :, 0:ch],
            op=ALU.is_equal,
        )
        w19a = ohpool.tile([128, HI * CHMAX], BF16, tag="w19a")
        w19a_v = w19a[:].rearrange("p (h t) -> p h t", t=CHMAX)
        nc.vector.tensor_tensor(
            out=w19a_v[:, :, 0:ch],
            in0=w19_v[:, :, 0:ch],
            in1=attn[:, :ch]
            .rearrange("p (o t) -> p o t", o=1)
            .to_broadcast([128, HI, ch]),
            op=ALU.mult,
        )
        for tt in range(ch):
            t = t0 + tt
            g = t // 2
            if g < Gb:
                bk = g // GPB
                if bk not in u_tiles:
                    u_tiles[bk] = upool.tile(
                        [HI, GPB * LO], F32, tag="u", name=f"u{bk}"
                    )
                o = u_tiles[bk][:, (g % GPB) * LO : (g % GPB) * LO + LO]
                st = t % 2 == 0
                sp = t % 2 == 1
            else:
                if "uF" not in u_tiles:
                    u_tiles["uF"] = upool.tile([HI, LO], F32, tag="u", name="uF")
                o = u_tiles["uF"][:]
                st = t == 2 * Gb
                sp = t == NT - 1
            nc.tensor.matmul(
                out=o, lhsT=w19a_v[:, :, tt], rhs=oh_v[:, :, tt], start=st, stop=sp
            )
        tend = t0 + ch
        while state["banks_done"] < NBb and 2 * min(
            (state["banks_done"] + 1) * GPB, Gb
        ) <= tend:
            b = state["banks_done"]
            done = 2 * min((b + 1) * GPB, Gb)
            # the LAST bank's chain is the tail's critical path: schedule its
            # copy immediately and its round-trip one step later
            d = 0 if b == NBb - 1 else 16
            pending_copy.append((done + d, b))
            state["banks_done"] += 1
        while pending_copy and pending_copy[0][0] <= tend:
            due, b = pending_copy.pop(0)
            pending_rt.append((due + 16, b, finalize_bank_copy(b)))
        while pending_rt and pending_rt[0][0] <= tend:
            due, b, usb_t = pending_rt.pop(0)
            pending_mms.append((due + 24, finalize_bank_rt(b, usb_t)))
        while pending_mms and pending_mms[0][0] <= tend:
            pending_mms.pop(0)[1]()

    # ---- main stream (software-pipelined emission) ----
    t0 = 0
    prev = None
    pending_copy = []
    pending_rt = []
    pending_mms = []
    for ch in sizes:
        scores = emit_front(t0, ch)
        if prev is not None:
            emit_rest(*prev)
        prev = (t0, ch, scores)
        t0 += ch
    emit_rest(*prev)
    assert state["banks_done"] == NBb, (state["banks_done"], NBb)
    # leftover bank round-trips (their group data is long since ready)
    while pending_copy:
        _, b = pending_copy.pop(0)
        pending_rt.append((0, b, finalize_bank_copy(b)))
    while pending_rt:
        _, b, usb_t = pending_rt.pop(0)
        pending_mms.append((0, finalize_bank_rt(b, usb_t)))
    if F > 0:
        finalize_fast()
    # leftover Sel matmuls go AFTER the fast-path matmuls: their xp read may
    # still be in flight and a parked matmul head-blocks the PE queue
    while pending_mms:
        pending_mms.pop(0)[1]()

    # close both PSUM accumulation groups over their FULL regions (partial
    # sub-region stops don't end the group for the untouched rows)
    nc.tensor.matmul(
        out=A_ps[:], lhsT=zrow8[:], rhs=zwide[:, 0 : 13 * LO], start=False, stop=True
    )
    nc.tensor.matmul(
        out=B_ps[:], lhsT=zrow8[:], rhs=zwide[:, 0 : 6 * LO + 1], start=False, stop=True
    )

    # ---- finalize: invZ, then logits = Ln(u*invZ + eps) straight from PSUM ----
    zz = sb.tile([BL, 1], F32)
    nc.vector.reciprocal(out=zz[:], in_=B_ps[:, 6 * LO : 6 * LO + 1])
    lg = sb.tile([BL, OUTE], F32)
    nc.scalar.activation(
        out=lg[:, 0:416].rearrange("j (h l) -> j h l", h=13),
        in_=A_ps[:].rearrange("j (h l) -> j h l", h=13)[:, :, 0:32],
        func=AF.Ln,
        bias=epscol[:, 0:1],
        scale=zz[:, 0:1],
    )
    nc.scalar.activation(
        out=lg[:, 416:512].rearrange("j (h l) -> j h l", h=3),
        in_=B_ps[:, 0 : 6 * LO].rearrange("j (h l) -> j h l", h=6)[:, 0:3, 0:32],
        func=AF.Ln,
        bias=epscol[:, 0:1],
        scale=zz[:, 0:1],
    )
    nc.sync.dma_start(out=out, in_=lg[:])


def build_program(NT, F):
    Gn = NT // 2
    Gb = Gn - F
    NBb = (Gb + GPB - 1) // GPB
    nc = bacc.Bacc(
        "TRN2",
        target_bir_lowering=False,
        debug=False,
        enable_asserts=False,
        num_devices=1,
    )
    docp = nc.dram_tensor(
        "docp", [128, NT + NT * 128], F16, kind="ExternalInput"
    ).ap()  # cols [0:NT] = per-tile q values; [NT:] = packed docT
    ids_d = nc.dram_tensor("ids", [128, 2 * NT], I16, kind="ExternalInput").ap()
    selb_d = nc.dram_tensor(
        "selb", [GPB, max(NBb, 1) * BL], BF16, kind="ExternalInput"
    ).ap()
    eye_d = nc.dram_tensor("eye19", [HI, HI], BF16, kind="ExternalInput").ap()
    xdram = nc.dram_tensor(
        "xdram", [max(Gb, 1), HI * LO], BF16, kind="ExternalInput"
    ).ap()
    out = nc.dram_tensor("out", [BL, OUTE], F32, kind="ExternalOutput").ap()

    tensors = (out, docp, ids_d, selb_d, eye_d, xdram)
    with tile.TileContext(nc) as tc:
        with ExitStack() as ctx:
            emit_kernel(ctx, tc, NT, F, tensors)
    nc.compile()
    return nc


def make_in_maps(doc_emb, query_emb, doc_ids, seq_length, plan):
    NT = plan["NT"]
    F = plan["F"]
    Gn = NT // 2
    Gb = Gn - F
    NBb = (Gb + GPB - 1) // GPB
    gt = plan["gt"]
    L = plan["L"]
    eye = np.eye(HI, dtype=np.float32).astype(BF16NP)
    in_maps = []
    for c in range(NCORES):
        bs = plan["assign"][c]
        docq = np.zeros((128, NT + NT * 128), np.float16)
        docT = docq[:, NT:].reshape(128, NT, 128)
        qcols = docq[:, :NT]
        ids2 = np.full((128, 2 * NT), -1, np.int16)
        idlo = ids2[:, :NT]
        idhi = ids2[:, NT:]
        selb = np.zeros((GPB, max(NBb, 1) * BL), BF16NP)
        t0 = 0
        p = np.arange(128)
        for j, b in enumerate(bs):
            nt = int(gt[b])
            lj = int(L[b])
            npos = min(nt * 128, S)
            seg = np.zeros((nt * 128, E), np.float32)
            seg[:npos] = doc_emb[b, :npos, :]
            docT[:, t0 : t0 + nt, :] = (
                seg.reshape(nt, 128, E).transpose(2, 0, 1).astype(np.float16)
            )
            qcols[:, t0 : t0 + nt] = query_emb[b].astype(np.float16)[:, None]
            svals = (np.arange(nt) * 128)[None, :] + p[:, None]
            valid = svals < lj
            idseg = np.zeros(nt * 128, np.int32)
            idseg[:npos] = doc_ids[b, :npos]
            idseg = idseg.reshape(nt, 128).T
            idlo[:, t0 : t0 + nt] = np.where(valid, idseg & 31, -1).astype(np.int16)
            idhi[:, t0 : t0 + nt] = np.where(valid, idseg >> 5, -1).astype(np.int16)
            slot = (j + 1) % BL  # stream-last batch -> slot 0 (fast path)
            for g in range(t0 // 2, (t0 + nt) // 2):
                if g < Gb:
                    selb[g % GPB, (g // GPB) * BL + slot] = 1.0
                else:
                    # fast groups are added straight into slot 0
                    assert slot == 0, (c, j, g, Gb)
            t0 += nt
        in_maps.append(
            {
                "docp": docq,
                "ids": ids2,
                "selb": selb,
                "eye19": eye,
                "xdram": np.zeros((max(Gb, 1), HI * LO), BF16NP),
            }
        )
    return in_maps


_CACHE = {}


def _get_program(key=None):
    if key is None:
        key = _CACHE.get("last_key")
        assert key is not None, "no program built yet"
    if key not in _CACHE:
        _CACHE[key] = build_program(*key)
    _CACHE["last_key"] = key
    return _CACHE[key]


def kernel(**inputs):
    doc_emb = np.asarray(inputs["doc_emb"], dtype=np.float32)
    query_emb = np.asarray(inputs["query_emb"], dtype=np.float32)
    doc_ids = np.asarray(inputs["doc_ids"], dtype=np.int32)
    seq_length = np.asarray(inputs["seq_length"], dtype=np.int32)

    plan = make_plan(seq_length)
    nc = _get_program((plan["NT"], plan["F"]))
    in_maps = make_in_maps(doc_emb, query_emb, doc_ids, seq_length, plan)
    res = bass_utils.run_bass_kernel_spmd(nc, in_maps, core_ids=list(range(NCORES)))
    out = np.zeros((B, OUTE), np.float32)
    for c in range(NCORES):
        o = np.asarray(res.results[c]["out"], dtype=np.float32)
        for j, b in enumerate(plan["assign"][c]):
            out[b] = o[(j + 1) % BL]
    return out


# revision 50
# speedup vs baseline: 1.0406x; 1.0092x over previous
"""Trainium2 Bass kernel for nn_AttentionSumReader (segment_reduce).

Pipeline per batch (B=64, S=4096, E=128, 600 entities -> logits over first 512):
  scores = doc_emb @ query          (per-batch matvec)
  attn   = masked softmax(scores)   (mask: s < max(seq_length,1))
  sums   = segment_sum(attn, doc_ids)[:512]
  out    = log(sums + 1e-9)

Strategy (v3 — JIT length-specialized flat tile stream + fast-path tail):
  - Data-parallel over batch: 8 batches/core, LOAD-BALANCED across cores by
    valid length (seq_length known on host pre-compile); only the valid
    prefix of each batch is streamed. Host pre-transposes doc to [E, s]
    f16 and packs per-core tile streams (tile = 128 positions). Programs
    are compiled per realized (NT, F) and cached. ~2x traffic from
    f32->f16, ~1.7x from skipping invalid positions, no on-chip transpose.
  - Per-tile batch context is data, not control flow: host sends per-tile
    query columns, additive masks, id hi/lo (int16), so one SPMD program
    serves per-core variable batch boundaries.
  - Matvec: docT tile stationary, per-tile q column moving; scores [128,ch]
    in PSUM. attn = exp(s) via exp(s/4)^4 (ACT), -2000 additive mask;
    normalization deferred (logits need u/Z once at the end).
  - Segment-sum: id = hi*32+lo (hi<19, lo<32). One-hots on DVE in 2-byte
    2x mode (l-major layout, int16 ids vs int16 iota -> bf16). lo-one-hot
    col 32 is constant 1 so each u block's col 32 accumulates per-hi attn
    sums (gives Z). Per-tile matmul (lhsT=w_hi*attn [128,19],
    rhs=oh_lo [128,33]) accumulates per-GROUP u[19,33] in PSUM (group=2
    tiles); batches own whole groups.
  - Group->batch reduction, two paths into per-batch PSUM A[8,429]/B[8,198]
    (= [8, hi, 33] split at hi 13):
    * banked groups (all but last F): u banks [19, 15*33] -> bf16 (ACT) ->
      DRAM -> reload with groups on partitions -> Sel[g, slot] matmul
      (Sel host-built per core). Round-trips overlap the stream (SWDGE
      queue, keeps HWDGE free for the doc stream).
    * fast groups (last F, host guarantees they belong to slot 7 = the
      longest batch, or are zero-mass dummies): u[19,33] -> bf16 -> 19
      identity-column matmuls add rows straight into A/B[7] - no DRAM
      round-trip on the critical tail.
  - Finalize: Z_j = sum_hi A/B[j, hi*33+32]; logits = Ln(u*invZ + eps).
"""

import sys

sys.path.insert(0, "/opt/trn_rl_repo")

from contextlib import ExitStack

import numpy as np
import ml_dtypes

from concourse import bacc, bass, mybir, tile
from concourse import bass_utils

BF16NP = ml_dtypes.bfloat16

# ---- problem constants (hardcoded; kernel.py must be self-contained) ----
B, S, E = 64, 4096, 128
NCORES = 8
BL = B // NCORES  # batches per core
LO = 33  # 32 lo values + 1 ones-column (for Z)
HI = 19  # 600 entities <= 19*32
GPB = 15  # groups per PSUM bank: 15*33*4B = 1980 <= 2048
CHMAX = 16  # max tiles per processing chunk
FMAX = 64  # max fast-path (no-roundtrip) tail groups
OUTE = 512
EPS = 1e-9

F32 = mybir.dt.float32
F16 = mybir.dt.float16
BF16 = mybir.dt.bfloat16
I16 = mybir.dt.int16

ALU = mybir.AluOpType
AF = mybir.ActivationFunctionType
AX = mybir.AxisListType


def make_plan(seq_length):
    """Balance batches across cores by padded valid-tile count; derive the
    uniform per-core stream length NT, fast-group count F, chunk split."""
    L = np.maximum(np.asarray(seq_length, dtype=np.int64), 1)
    tiles = (L + 127) // 128
    gt = 2 * ((tiles + 1) // 2)  # pad each batch to whole groups (G=2)
    order = np.argsort(-gt, kind="stable")
    loads = [0] * NCORES
    counts = [0] * NCORES
    assign = [[] for _ in range(NCORES)]
    for b in order:
        c = min(
            (i for i in range(NCORES) if counts[i] < BL), key=lambda i: loads[i]
        )
        loads[c] += int(gt[b])
        counts[c] += 1
        assign[c].append(int(b))
    # slot order ascending by length: slot BL-1 (stream-last) = longest batch
    assign = [list(reversed(a)) for a in assign]
    NT = int(max(loads))
    NT = max(NT, 4)
    if NT % 2:
        NT += 1
    Gn = NT // 2
    # fast groups must contain only slot-7 tiles or dummies on EVERY core
    start_g7 = []
    for c in range(NCORES):
        pre = sum(int(gt[b]) for b in assign[c][: BL - 1])
        start_g7.append(pre // 2)
    F = max(0, min(FMAX, Gn - max(start_g7)))
    return {"assign": assign, "gt": gt, "L": L, "NT": NT, "F": F}


def make_chunk_sizes(NT, Gb):
    """Even-sized chunks covering [0, 2*Gb) then [2*Gb, NT): a chunk boundary
    lands exactly on the banked/fast split so the last bank's DRAM round-trip
    starts as early as possible. Small first chunk for a fast pipeline head."""

    def split(n, first=None):
        sizes = []
        if first and n >= first:
            sizes.append(first)
            n -= first
        while n > CHMAX:
            sizes.append(CHMAX)
            n -= CHMAX
        if n:
            sizes.append(n)
        return sizes

    return split(2 * Gb, first=4), split(NT - 2 * Gb)


def emit_kernel(ctx, tc, NT, F, tensors):
    nc = tc.nc
    Gn = NT // 2
    Gb = Gn - F  # banked groups
    NBb = (Gb + GPB - 1) // GPB
    sizes_a, sizes_b = make_chunk_sizes(NT, Gb)
    sizes = sizes_a + sizes_b

    (out, docp, ids_d, selb_d, eye_d, xdram) = tensors

    sb = ctx.enter_context(tc.tile_pool(name="sb", bufs=1))
    docpool = ctx.enter_context(tc.tile_pool(name="docp", bufs=6))
    wkpool = ctx.enter_context(tc.tile_pool(name="wk", bufs=3))
    ohpool = ctx.enter_context(tc.tile_pool(name="oh", bufs=3))
    usbpool = ctx.enter_context(tc.tile_pool(name="usb", bufs=2))
    ufpool = ctx.enter_context(tc.tile_pool(name="ufp", bufs=1))
    xppool = ctx.enter_context(tc.tile_pool(name="xp", bufs=max(NBb, 1)))
    scpool = ctx.enter_context(tc.tile_pool(name="sc", bufs=3, space="PSUM"))
    upool = ctx.enter_context(tc.tile_pool(name="up", bufs=3, space="PSUM"))
    abpool = ctx.enter_context(tc.tile_pool(name="ab", bufs=1, space="PSUM"))

    # ---- small inputs ----
    iota33_t = sb.tile([128, LO * CHMAX], I16)
    nc.gpsimd.iota(
        iota33_t[:], pattern=[[1, LO], [0, CHMAX]], base=0, channel_multiplier=0
    )
    iota19_t = sb.tile([128, HI * CHMAX], I16)
    nc.gpsimd.iota(
        iota19_t[:], pattern=[[1, HI], [0, CHMAX]], base=0, channel_multiplier=0
    )
    iota33 = iota33_t[:]
    iota19 = iota19_t[:]
    # ACT/HWDGE queue: ids in one small DMA (transfers are tiny)
    ids_t = sb.tile([128, 2 * NT], I16)
    nc.scalar.dma_start(out=ids_t[:], in_=ids_d)
    idlo = ids_t[:, 0:NT]
    idhi = ids_t[:, NT : 2 * NT]
    selb = sb.tile([GPB, max(NBb, 1) * BL], BF16)
    if NBb:
        nc.gpsimd.dma_start(out=selb[:], in_=selb_d)
    eye19 = sb.tile([HI, HI], BF16)
    nc.gpsimd.dma_start(out=eye19[:], in_=eye_d)
    zcol = sb.tile([128, 1], F32)
    nc.vector.memset(zcol[:], 0.0)
    epscol = sb.tile([BL, 1], F32)
    nc.vector.memset(epscol[:], EPS)
    ones19 = sb.tile([HI, 1], BF16)
    nc.vector.memset(ones19[:], 1.0)
    zrow8 = sb.tile([1, BL], BF16)
    nc.vector.memset(zrow8[:], 0.0)
    zwide = sb.tile([1, 13 * LO], BF16)
    nc.vector.memset(zwide[:], 0.0)
    # pin the combined exp+ln activation table once, up front: the auto
    # table-load pass then inserts nothing (no 1.3us reload before the final
    # Ln on the critical tail)
    from concourse.hw_specs import get_activation_tables

    tabs = list(get_activation_tables(nc.m.arch).items())
    combined = next(
        i for i, (k, v) in enumerate(tabs) if AF.Exp in v and AF.Ln in v
    )
    nc.scalar.add_instruction(
        mybir.InstLoadActFuncSet(
            name=nc.get_next_instruction_name(),
            act_func_set_id=combined,
            ins=[],
            outs=[],
        )
    )

    u_tiles = {}
    A_ps = abpool.tile([BL, 13 * LO], F32, tag="A")
    B_ps = abpool.tile([BL, 6 * LO + 1], F32, tag="B")  # col 198 = Z
    # A/B accumulation bookkeeping: first writer starts, closers stop.
    a_seen = [0]
    b_seen = [0]

    def finalize_bank_copy(b):
        gl = min(GPB, Gb - b * GPB)
        usb_t = usbpool.tile([HI, GPB * LO], BF16, tag="usb")
        nc.vector.tensor_copy(out=usb_t[:, : gl * LO], in_=u_tiles[b][:, : gl * LO])
        return usb_t

    def finalize_bank_rt(b, usb_t):
        gl = min(GPB, Gb - b * GPB)
        # write on the SP queue (its wait is long satisfied; the doc queue has
        # ~800ns/chunk of issue slack); read alone on the Pool queue, where
        # its ~2.3us wait on the write's completion blocks nothing else
        nc.sync.dma_start(
            out=xdram[b * GPB : b * GPB + gl, :].rearrange("g (h l) -> h g l", h=HI),
            in_=usb_t[:, : gl * LO].rearrange("h (g l) -> h g l", g=gl),
        )
        xp_t = xppool.tile([GPB, HI * LO + 1], BF16, tag="xp")
        nc.gpsimd.dma_start(
            out=xp_t[:gl, 0 : HI * LO], in_=xdram[b * GPB : b * GPB + gl, :]
        )

        gl = min(GPB, Gb - b * GPB)

        def mms():
            # per-group total attn (for Z): sum the 19 col-32 entries per row
            with nc.allow_low_precision(reason="Z column; rel err ~2^-8 on ln(Z)"):
                nc.vector.tensor_reduce(
                    out=xp_t[:gl, HI * LO : HI * LO + 1],
                    in_=xp_t[:gl, 0 : HI * LO].rearrange("g (h l) -> g h l", l=LO)[
                        :, :, 32
                    ],
                    axis=AX.X,
                    op=ALU.add,
                )
            a_seen[0] += 1
            # the last bank's Sel matmuls are emitted last among all A/B
            # writers and cover the FULL regions, so they close the psum
            # accumulation groups (no separate zero-closers needed)
            nc.tensor.matmul(
                out=A_ps[:],
                lhsT=selb[0:gl, b * BL : (b + 1) * BL],
                rhs=xp_t[0:gl, 0 : 13 * LO],
                start=(a_seen[0] == 1),
                stop=(b == NBb - 1),
            )
            b_seen[0] += 1
            nc.tensor.matmul(
                out=B_ps[:],
                lhsT=selb[0:gl, b * BL : (b + 1) * BL],
                rhs=xp_t[0:gl, 13 * LO : HI * LO + 1],
                start=(b_seen[0] == 1),
                stop=(b == NBb - 1),
            )

        return mms

    def finalize_fast():
        # the whole fast region belongs to the stream-last batch, which the
        # host maps to SLOT 0 (PE matmul out base partition must be 0/32/64);
        # it was accumulated into the single uF psum tile. One bf16 copy, then
        # 19 identity-column matmuls scatter its rows into A/B[0], plus one
        # ones-column matmul for its Z contribution.
        uf_sb = ufpool.tile([HI, LO], BF16, tag="uf")
        nc.vector.tensor_copy(out=uf_sb[:], in_=u_tiles["uF"][:])
        a_seen[0] += 1
        b_seen[0] += 1
        for h in range(HI):
            if h < 13:
                o = A_ps[0:1, h * LO : (h + 1) * LO]
                st = a_seen[0] == 1 and h == 0
            else:
                o = B_ps[0:1, (h - 13) * LO : (h - 12) * LO]
                st = b_seen[0] == 1 and h == 13
            nc.tensor.matmul(
                out=o, lhsT=eye19[:, h : h + 1], rhs=uf_sb[:], start=st, stop=False
            )
        nc.tensor.matmul(
            out=B_ps[0:1, 6 * LO : 6 * LO + 1],
            lhsT=ones19[:],
            rhs=uf_sb[:, 32:33],
            start=False,
            stop=False,
        )

    state = {"banks_done": 0}
    qcols_sb = sb.tile([128, NT], F16)

    def emit_front(t0, ch):
        """doc DMA + matvecs: emitted a chunk ahead so the PE queue never
        head-blocks the next chunk's matvecs behind this chunk's segmats."""
        doc_t = docpool.tile([128, CHMAX * 128], F16, tag="doc")
        nc.sync.dma_start(
            out=doc_t[:, : ch * 128],
            in_=docp[:, NT + t0 * 128 : NT + (t0 + ch) * 128],
        )
        if t0 == 0:
            nc.scalar.dma_start(out=qcols_sb[:], in_=docp[:, 0:NT])
        scores = scpool.tile([128, CHMAX], F32, tag="sc")
        for tt in range(ch):
            nc.tensor.matmul(
                out=scores[:, tt : tt + 1],
                lhsT=doc_t[:, tt * 128 : (tt + 1) * 128],
                rhs=qcols_sb[:, t0 + tt : t0 + tt + 1],
                start=True,
                stop=True,
            )
        return scores

    def emit_rest(t0, ch, scores):
        # attn = exp(s) straight from PSUM in bf16 (s <= ~83 < ln(bf16 max));
        # masking is free: hosts sets ids=-1 at invalid positions, so both
        # one-hots (incl. the is_ge Z column) are all-zero there
        attn = wkpool.tile([128, CHMAX], BF16, tag="attn")
        nc.scalar.activation(
            out=attn[:, :ch], in_=scores[:, :ch], func=AF.Exp, bias=zcol[:, 0:1], scale=1.0
        )
        oh_t = ohpool.tile([128, LO * CHMAX], BF16, tag="oh")
        oh_v = oh_t[:].rearrange("p (l t) -> p l t", t=CHMAX)
        nc.vector.tensor_tensor(
            out=oh_v[:, :, 0:ch],
            in0=idlo[:, t0 : t0 + ch]
            .rearrange("p (o t) -> p o t", o=1)
            .to_broadcast([128, LO, ch]),
            in1=iota33.rearrange("p (l t) -> p l t", t=CHMAX)[:, :, 0:ch],
            op=ALU.is_equal,
        )
        nc.vector.tensor_scalar(
            out=oh_t[:, 32 * CHMAX : 32 * CHMAX + ch],
            in0=idlo[:, t0 : t0 + ch],
            scalar1=0,
            scalar2=None,
            op0=ALU.is_ge,
        )
        w19 = ohpool.tile([128, HI * CHMAX], BF16, tag="w19")
        w19_v = w19[:].rearrange("p (h t) -> p h t", t=CHMAX)
        nc.vector.tensor_tensor(
            out=w19_v[:, :, 0:ch],
            in0=idhi[:, t0 : t0 + ch]
            .rearrange("p (o t) -> p o t", o=1)
            .to_broadcast([128, HI, ch]),
            in1=iota19.rearrange("p (h t) -> p h t", t=CHMAX)[:, :, 0:ch],
            op=ALU.is_equal,
        )
        w19a = ohpool.tile([128, HI * CHMAX], BF16, tag="w19a")
        w19a_v = w19a[:].rearrange("p (h t) -> p h t", t=CHMAX)
        nc.vector.tensor_tensor(
            out=w19a_v[:, :, 0:ch],
            in0=w19_v[:, :, 0:ch],
            in1=attn[:, :ch]
            .rearrange("p (o t) -> p o t", o=1)
            .to_broadcast([128, HI, ch]),
            op=ALU.mult,
        )
        for tt in range(ch):
            t = t0 + tt
            g = t // 2
            if g < Gb:
                bk = g // GPB
                if bk not in u_tiles:
                    u_tiles[bk] = upool.tile(
                        [HI, GPB * LO], F32, tag="u", name=f"u{bk}"
                    )
                o = u_tiles[bk][:, (g % GPB) * LO : (g % GPB) * LO + LO]
                st = t % 2 == 0
                sp = t % 2 == 1
            else:
                if "uF" not in u_tiles:
                    u_tiles["uF"] = upool.tile([HI, LO], F32, tag="u", name="uF")
                o = u_tiles["uF"][:]
                st = t == 2 * Gb
                sp = t == NT - 1
            nc.tensor.matmul(
                out=o, lhsT=w19a_v[:, :, tt], rhs=oh_v[:, :, tt], start=st, stop=sp
            )
        tend = t0 + ch
        while state["banks_done"] < NBb and 2 * min(
            (state["banks_done"] + 1) * GPB, Gb
        ) <= tend:
            b = state["banks_done"]
            done = 2 * min((b + 1) * GPB, Gb)
            # the LAST bank's chain is the tail's critical path: schedule its
            # copy immediately and its round-trip one step later
            d = 0 if b == NBb - 1 else 16
            pending_copy.append((done + d, b))
            state["banks_done"] += 1
        while pending_copy and pending_copy[0][0] <= tend:
            due, b = pending_copy.pop(0)
            pending_rt.append((due + 16, b, finalize_bank_copy(b)))
        while pending_rt and pending_rt[0][0] <= tend:
            due, b, usb_t = pending_rt.pop(0)
            pending_mms.append((due + 24, finalize_bank_rt(b, usb_t)))
        while pending_mms and pending_mms[0][0] <= tend:
            pending_mms.pop(0)[1]()

    # ---- main stream (software-pipelined emission) ----
    t0 = 0
    prev = None
    pending_copy = []
    pending_rt = []
    pending_mms = []
    for ch in sizes:
        scores = emit_front(t0, ch)
        if prev is not None:
            emit_rest(*prev)
        prev = (t0, ch, scores)
        t0 += ch
    emit_rest(*prev)
    assert state["banks_done"] == NBb, (state["banks_done"], NBb)
    # leftover bank round-trips (their group data is long since ready)
    while pending_copy:
        _, b = pending_copy.pop(0)
        pending_rt.append((0, b, finalize_bank_copy(b)))
    while pending_rt:
        _, b, usb_t = pending_rt.pop(0)
        pending_mms.append((0, finalize_bank_rt(b, usb_t)))
    if F > 0:
        finalize_fast()
    # leftover Sel matmuls go AFTER the fast-path matmuls: their xp read may
    # still be in flight and a parked matmul head-blocks the PE queue
    while pending_mms:
        pending_mms.pop(0)[1]()

    if NBb == 0:
        # no bank matmuls: close the accumulation groups with zero matmuls
        # (the uF identity matmuls only touch sub-regions of row 0)
        nc.tensor.matmul(
            out=A_ps[:], lhsT=zrow8[:], rhs=zwide[:, 0 : 13 * LO], start=False, stop=True
        )
        nc.tensor.matmul(
            out=B_ps[:], lhsT=zrow8[:], rhs=zwide[:, 0 : 6 * LO + 1], start=False, stop=True
        )

    # ---- finalize: invZ, then logits = Ln(u*invZ + eps) straight from PSUM ----
    zz = sb.tile([BL, 1], F32)
    nc.vector.reciprocal(out=zz[:], in_=B_ps[:, 6 * LO : 6 * LO + 1])
    lg = sb.tile([BL, OUTE], F32)
    nc.scalar.activation(
        out=lg[:, 0:416].rearrange("j (h l) -> j h l", h=13),
        in_=A_ps[:].rearrange("j (h l) -> j h l", h=13)[:, :, 0:32],
        func=AF.Ln,
        bias=epscol[:, 0:1],
        scale=zz[:, 0:1],
    )
    nc.scalar.activation(
        out=lg[:, 416:512].rearrange("j (h l) -> j h l", h=3),
        in_=B_ps[:, 0 : 6 * LO].rearrange("j (h l) -> j h l", h=6)[:, 0:3, 0:32],
        func=AF.Ln,
        bias=epscol[:, 0:1],
        scale=zz[:, 0:1],
    )
    nc.sync.dma_start(out=out, in_=lg[:])


def build_program(NT, F):
    Gn = NT // 2
    Gb = Gn - F
    NBb = (Gb + GPB - 1) // GPB
    nc = bacc.Bacc(
        "TRN2",
        target_bir_lowering=False,
        debug=False,
        enable_asserts=False,
        num_devices=1,
    )
    docp = nc.dram_tensor(
        "docp", [128, NT + NT * 128], F16, kind="ExternalInput"
    ).ap()  # cols [0:NT] = per-tile q values; [NT:] = packed docT
    ids_d = nc.dram_tensor("ids", [128, 2 * NT], I16, kind="ExternalInput").ap()
    selb_d = nc.dram_tensor(
        "selb", [GPB, max(NBb, 1) * BL], BF16, kind="ExternalInput"
    ).ap()
    eye_d = nc.dram_tensor("eye19", [HI, HI], BF16, kind="ExternalInput").ap()
    xdram = nc.dram_tensor(
        "xdram", [max(Gb, 1), HI * LO], BF16, kind="ExternalInput"
    ).ap()
    out = nc.dram_tensor("out", [BL, OUTE], F32, kind="ExternalOutput").ap()

    tensors = (out, docp, ids_d, selb_d, eye_d, xdram)
    with tile.TileContext(nc) as tc:
        with ExitStack() as ctx:
            emit_kernel(ctx, tc, NT, F, tensors)
    nc.compile()
    return nc


def make_in_maps(doc_emb, query_emb, doc_ids, seq_length, plan):
    NT = plan["NT"]
    F = plan["F"]
    Gn = NT // 2
    Gb = Gn - F
    NBb = (Gb + GPB - 1) // GPB
    gt = plan["gt"]
    L = plan["L"]
    eye = np.eye(HI, dtype=np.float32).astype(BF16NP)
    in_maps = []
    for c in range(NCORES):
        bs = plan["assign"][c]
        docq = np.zeros((128, NT + NT * 128), np.float16)
        docT = docq[:, NT:].reshape(128, NT, 128)
        qcols = docq[:, :NT]
        ids2 = np.full((128, 2 * NT), -1, np.int16)
        idlo = ids2[:, :NT]
        idhi = ids2[:, NT:]
        selb = np.zeros((GPB, max(NBb, 1) * BL), BF16NP)
        t0 = 0
        p = np.arange(128)
        for j, b in enumerate(bs):
            nt = int(gt[b])
            lj = int(L[b])
            npos = min(nt * 128, S)
            seg = np.zeros((nt * 128, E), np.float32)
            seg[:npos] = doc_emb[b, :npos, :]
            docT[:, t0 : t0 + nt, :] = (
                seg.reshape(nt, 128, E).transpose(2, 0, 1).astype(np.float16)
            )
            qcols[:, t0 : t0 + nt] = query_emb[b].astype(np.float16)[:, None]
            svals = (np.arange(nt) * 128)[None, :] + p[:, None]
            valid = svals < lj
            idseg = np.zeros(nt * 128, np.int32)
            idseg[:npos] = doc_ids[b, :npos]
            idseg = idseg.reshape(nt, 128).T
            idlo[:, t0 : t0 + nt] = np.where(valid, idseg & 31, -1).astype(np.int16)
            idhi[:, t0 : t0 + nt] = np.where(valid, idseg >> 5, -1).astype(np.int16)
            slot = (j + 1) % BL  # stream-last batch -> slot 0 (fast path)
            for g in range(t0 // 2, (t0 + nt) // 2):
                if g < Gb:
                    selb[g % GPB, (g // GPB) * BL + slot] = 1.0
                else:
                    # fast groups are added straight into slot 0
                    assert slot == 0, (c, j, g, Gb)
            t0 += nt
        in_maps.append(
            {
                "docp": docq,
                "ids": ids2,
                "selb": selb,
                "eye19": eye,
                "xdram": np.zeros((max(Gb, 1), HI * LO), BF16NP),
            }
        )
    return in_maps


_CACHE = {}


def _get_program(key=None):
    if key is None:
        key = _CACHE.get("last_key")
        assert key is not None, "no program built yet"
    if key not in _CACHE:
        _CACHE[key] = build_program(*key)
    _CACHE["last_key"] = key
    return _CACHE[key]


def kernel(**inputs):
    doc_emb = np.asarray(inputs["doc_emb"], dtype=np.float32)
    query_emb = np.asarray(inputs["query_emb"], dtype=np.float32)
    doc_ids = np.asarray(inputs["doc_ids"], dtype=np.int32)
    seq_length = np.asarray(inputs["seq_length"], dtype=np.int32)

    plan = make_plan(seq_length)
    nc = _get_program((plan["NT"], plan["F"]))
    in_maps = make_in_maps(doc_emb, query_emb, doc_ids, seq_length, plan)
    res = bass_utils.run_bass_kernel_spmd(nc, in_maps, core_ids=list(range(NCORES)))
    out = np.zeros((B, OUTE), np.float32)
    for c in range(NCORES):
        o = np.asarray(res.results[c]["out"], dtype=np.float32)
        for j, b in enumerate(plan["assign"][c]):
            out[b] = o[(j + 1) % BL]
    return out


# revision 51
# speedup vs baseline: 1.0425x; 1.0018x over previous
"""Trainium2 Bass kernel for nn_AttentionSumReader (segment_reduce).

Pipeline per batch (B=64, S=4096, E=128, 600 entities -> logits over first 512):
  scores = doc_emb @ query          (per-batch matvec)
  attn   = masked softmax(scores)   (mask: s < max(seq_length,1))
  sums   = segment_sum(attn, doc_ids)[:512]
  out    = log(sums + 1e-9)

Strategy (v3 — JIT length-specialized flat tile stream + fast-path tail):
  - Data-parallel over batch: 8 batches/core, LOAD-BALANCED across cores by
    valid length (seq_length known on host pre-compile); only the valid
    prefix of each batch is streamed. Host pre-transposes doc to [E, s]
    f16 and packs per-core tile streams (tile = 128 positions). Programs
    are compiled per realized (NT, F) and cached. ~2x traffic from
    f32->f16, ~1.7x from skipping invalid positions, no on-chip transpose.
  - Per-tile batch context is data, not control flow: host sends per-tile
    query columns, additive masks, id hi/lo (int16), so one SPMD program
    serves per-core variable batch boundaries.
  - Matvec: docT tile stationary, per-tile q column moving; scores [128,ch]
    in PSUM. attn = exp(s) via exp(s/4)^4 (ACT), -2000 additive mask;
    normalization deferred (logits need u/Z once at the end).
  - Segment-sum: id = hi*32+lo (hi<19, lo<32). One-hots on DVE in 2-byte
    2x mode (l-major layout, int16 ids vs int16 iota -> bf16). lo-one-hot
    col 32 is constant 1 so each u block's col 32 accumulates per-hi attn
    sums (gives Z). Per-tile matmul (lhsT=w_hi*attn [128,19],
    rhs=oh_lo [128,33]) accumulates per-GROUP u[19,33] in PSUM (group=2
    tiles); batches own whole groups.
  - Group->batch reduction, two paths into per-batch PSUM A[8,429]/B[8,198]
    (= [8, hi, 33] split at hi 13):
    * banked groups (all but last F): u banks [19, 15*33] -> bf16 (ACT) ->
      DRAM -> reload with groups on partitions -> Sel[g, slot] matmul
      (Sel host-built per core). Round-trips overlap the stream (SWDGE
      queue, keeps HWDGE free for the doc stream).
    * fast groups (last F, host guarantees they belong to slot 7 = the
      longest batch, or are zero-mass dummies): u[19,33] -> bf16 -> 19
      identity-column matmuls add rows straight into A/B[7] - no DRAM
      round-trip on the critical tail.
  - Finalize: Z_j = sum_hi A/B[j, hi*33+32]; logits = Ln(u*invZ + eps).
"""

import sys

sys.path.insert(0, "/opt/trn_rl_repo")

from contextlib import ExitStack

import numpy as np
import ml_dtypes

from concourse import bacc, bass, mybir, tile
from concourse import bass_utils

BF16NP = ml_dtypes.bfloat16

# ---- problem constants (hardcoded; kernel.py must be self-contained) ----
B, S, E = 64, 4096, 128
NCORES = 8
BL = B // NCORES  # batches per core
LO = 33  # 32 lo values + 1 ones-column (for Z)
HI = 19  # 600 entities <= 19*32
GPB = 15  # groups per PSUM bank: 15*33*4B = 1980 <= 2048
CHMAX = 16  # max tiles per processing chunk
FMAX = 64  # max fast-path (no-roundtrip) tail groups
OUTE = 512
EPS = 1e-9

F32 = mybir.dt.float32
F16 = mybir.dt.float16
BF16 = mybir.dt.bfloat16
I16 = mybir.dt.int16

ALU = mybir.AluOpType
AF = mybir.ActivationFunctionType
AX = mybir.AxisListType


def make_plan(seq_length):
    """Balance batches across cores by padded valid-tile count; derive the
    uniform per-core stream length NT, fast-group count F, chunk split."""
    L = np.maximum(np.asarray(seq_length, dtype=np.int64), 1)
    tiles = (L + 127) // 128
    gt = 2 * ((tiles + 1) // 2)  # pad each batch to whole groups (G=2)
    order = np.argsort(-gt, kind="stable")
    loads = [0] * NCORES
    counts = [0] * NCORES
    assign = [[] for _ in range(NCORES)]
    for b in order:
        c = min(
            (i for i in range(NCORES) if counts[i] < BL), key=lambda i: loads[i]
        )
        loads[c] += int(gt[b])
        counts[c] += 1
        assign[c].append(int(b))
    # slot order ascending by length: slot BL-1 (stream-last) = longest batch
    assign = [list(reversed(a)) for a in assign]
    NT = int(max(loads))
    NT = max(NT, 4)
    if NT % 2:
        NT += 1
    Gn = NT // 2
    # fast groups must contain only slot-7 tiles or dummies on EVERY core
    start_g7 = []
    for c in range(NCORES):
        pre = sum(int(gt[b]) for b in assign[c][: BL - 1])
        start_g7.append(pre // 2)
    F = max(0, min(FMAX, Gn - max(start_g7)))
    return {"assign": assign, "gt": gt, "L": L, "NT": NT, "F": F}


def make_chunk_sizes(NT, Gb):
    """Even-sized chunks covering [0, 2*Gb) then [2*Gb, NT): a chunk boundary
    lands exactly on the banked/fast split so the last bank's DRAM round-trip
    starts as early as possible. Small first chunk for a fast pipeline head."""

    def split(n, first=None):
        sizes = []
        if first and n >= first:
            sizes.append(first)
            n -= first
        while n > CHMAX:
            sizes.append(CHMAX)
            n -= CHMAX
        if n:
            sizes.append(n)
        return sizes

    return split(2 * Gb, first=4), split(NT - 2 * Gb)


def emit_kernel(ctx, tc, NT, F, tensors):
    nc = tc.nc
    Gn = NT // 2
    Gb = Gn - F  # banked groups
    NBb = (Gb + GPB - 1) // GPB
    sizes_a, sizes_b = make_chunk_sizes(NT, Gb)
    sizes = sizes_a + sizes_b

    (out, docp, ids_d, selb_d, eye_d, xdram) = tensors

    sb = ctx.enter_context(tc.tile_pool(name="sb", bufs=1))
    docpool = ctx.enter_context(tc.tile_pool(name="docp", bufs=8))
    wkpool = ctx.enter_context(tc.tile_pool(name="wk", bufs=3))
    ohpool = ctx.enter_context(tc.tile_pool(name="oh", bufs=3))
    usbpool = ctx.enter_context(tc.tile_pool(name="usb", bufs=2))
    ufpool = ctx.enter_context(tc.tile_pool(name="ufp", bufs=1))
    xppool = ctx.enter_context(tc.tile_pool(name="xp", bufs=max(NBb, 1)))
    scpool = ctx.enter_context(tc.tile_pool(name="sc", bufs=3, space="PSUM"))
    upool = ctx.enter_context(tc.tile_pool(name="up", bufs=3, space="PSUM"))
    abpool = ctx.enter_context(tc.tile_pool(name="ab", bufs=1, space="PSUM"))

    # ---- small inputs ----
    iota33_t = sb.tile([128, LO * CHMAX], I16)
    nc.gpsimd.iota(
        iota33_t[:], pattern=[[1, LO], [0, CHMAX]], base=0, channel_multiplier=0
    )
    iota19_t = sb.tile([128, HI * CHMAX], I16)
    nc.gpsimd.iota(
        iota19_t[:], pattern=[[1, HI], [0, CHMAX]], base=0, channel_multiplier=0
    )
    iota33 = iota33_t[:]
    iota19 = iota19_t[:]
    # ACT/HWDGE queue: ids in one small DMA (transfers are tiny)
    ids_t = sb.tile([128, 2 * NT], I16)
    nc.scalar.dma_start(out=ids_t[:], in_=ids_d)
    idlo = ids_t[:, 0:NT]
    idhi = ids_t[:, NT : 2 * NT]
    selb = sb.tile([GPB, max(NBb, 1) * BL], BF16)
    if NBb:
        nc.gpsimd.dma_start(out=selb[:], in_=selb_d)
    eye19 = sb.tile([HI, HI], BF16)
    nc.gpsimd.dma_start(out=eye19[:], in_=eye_d)
    zcol = sb.tile([128, 1], F32)
    nc.vector.memset(zcol[:], 0.0)
    epscol = sb.tile([BL, 1], F32)
    nc.vector.memset(epscol[:], EPS)
    ones19 = sb.tile([HI, 1], BF16)
    nc.vector.memset(ones19[:], 1.0)
    zrow8 = sb.tile([1, BL], BF16)
    nc.vector.memset(zrow8[:], 0.0)
    zwide = sb.tile([1, 13 * LO], BF16)
    nc.vector.memset(zwide[:], 0.0)
    # pin the combined exp+ln activation table once, up front: the auto
    # table-load pass then inserts nothing (no 1.3us reload before the final
    # Ln on the critical tail)
    from concourse.hw_specs import get_activation_tables

    tabs = list(get_activation_tables(nc.m.arch).items())
    combined = next(
        i for i, (k, v) in enumerate(tabs) if AF.Exp in v and AF.Ln in v
    )
    nc.scalar.add_instruction(
        mybir.InstLoadActFuncSet(
            name=nc.get_next_instruction_name(),
            act_func_set_id=combined,
            ins=[],
            outs=[],
        )
    )

    u_tiles = {}
    A_ps = abpool.tile([BL, 13 * LO], F32, tag="A")
    B_ps = abpool.tile([BL, 6 * LO + 1], F32, tag="B")  # col 198 = Z
    # A/B accumulation bookkeeping: first writer starts, closers stop.
    a_seen = [0]
    b_seen = [0]

    def finalize_bank_copy(b):
        gl = min(GPB, Gb - b * GPB)
        usb_t = usbpool.tile([HI, GPB * LO], BF16, tag="usb")
        nc.vector.tensor_copy(out=usb_t[:, : gl * LO], in_=u_tiles[b][:, : gl * LO])
        return usb_t

    def finalize_bank_rt(b, usb_t):
        gl = min(GPB, Gb - b * GPB)
        # write on the SP queue (its wait is long satisfied; the doc queue has
        # ~800ns/chunk of issue slack); read alone on the Pool queue, where
        # its ~2.3us wait on the write's completion blocks nothing else
        nc.sync.dma_start(
            out=xdram[b * GPB : b * GPB + gl, :].rearrange("g (h l) -> h g l", h=HI),
            in_=usb_t[:, : gl * LO].rearrange("h (g l) -> h g l", g=gl),
        )
        xp_t = xppool.tile([GPB, HI * LO + 1], BF16, tag="xp")
        nc.gpsimd.dma_start(
            out=xp_t[:gl, 0 : HI * LO], in_=xdram[b * GPB : b * GPB + gl, :]
        )

        gl = min(GPB, Gb - b * GPB)

        def mms():
            # per-group total attn (for Z): sum the 19 col-32 entries per row
            with nc.allow_low_precision(reason="Z column; rel err ~2^-8 on ln(Z)"):
                nc.vector.tensor_reduce(
                    out=xp_t[:gl, HI * LO : HI * LO + 1],
                    in_=xp_t[:gl, 0 : HI * LO].rearrange("g (h l) -> g h l", l=LO)[
                        :, :, 32
                    ],
                    axis=AX.X,
                    op=ALU.add,
                )
            a_seen[0] += 1
            # the last bank's Sel matmuls are emitted last among all A/B
            # writers and cover the FULL regions, so they close the psum
            # accumulation groups (no separate zero-closers needed)
            nc.tensor.matmul(
                out=A_ps[:],
                lhsT=selb[0:gl, b * BL : (b + 1) * BL],
                rhs=xp_t[0:gl, 0 : 13 * LO],
                start=(a_seen[0] == 1),
                stop=(b == NBb - 1),
            )
            b_seen[0] += 1
            nc.tensor.matmul(
                out=B_ps[:],
                lhsT=selb[0:gl, b * BL : (b + 1) * BL],
                rhs=xp_t[0:gl, 13 * LO : HI * LO + 1],
                start=(b_seen[0] == 1),
                stop=(b == NBb - 1),
            )

        return mms

    def finalize_fast():
        # the whole fast region belongs to the stream-last batch, which the
        # host maps to SLOT 0 (PE matmul out base partition must be 0/32/64);
        # it was accumulated into the single uF psum tile. One bf16 copy, then
        # 19 identity-column matmuls scatter its rows into A/B[0], plus one
        # ones-column matmul for its Z contribution.
        uf_sb = ufpool.tile([HI, LO], BF16, tag="uf")
        nc.vector.tensor_copy(out=uf_sb[:], in_=u_tiles["uF"][:])
        a_seen[0] += 1
        b_seen[0] += 1
        for h in range(HI):
            if h < 13:
                o = A_ps[0:1, h * LO : (h + 1) * LO]
                st = a_seen[0] == 1 and h == 0
            else:
                o = B_ps[0:1, (h - 13) * LO : (h - 12) * LO]
                st = b_seen[0] == 1 and h == 13
            nc.tensor.matmul(
                out=o, lhsT=eye19[:, h : h + 1], rhs=uf_sb[:], start=st, stop=False
            )
        nc.tensor.matmul(
            out=B_ps[0:1, 6 * LO : 6 * LO + 1],
            lhsT=ones19[:],
            rhs=uf_sb[:, 32:33],
            start=False,
            stop=False,
        )

    state = {"banks_done": 0}
    qcols_sb = sb.tile([128, NT], F16)

    def emit_front(t0, ch):
        """doc DMA + matvecs: emitted a chunk ahead so the PE queue never
        head-blocks the next chunk's matvecs behind this chunk's segmats."""
        doc_t = docpool.tile([128, CHMAX * 128], F16, tag="doc")
        nc.sync.dma_start(
            out=doc_t[:, : ch * 128],
            in_=docp[:, NT + t0 * 128 : NT + (t0 + ch) * 128],
        )
        if t0 == 0:
            nc.scalar.dma_start(out=qcols_sb[:], in_=docp[:, 0:NT])
        scores = scpool.tile([128, CHMAX], F32, tag="sc")
        for tt in range(ch):
            nc.tensor.matmul(
                out=scores[:, tt : tt + 1],
                lhsT=doc_t[:, tt * 128 : (tt + 1) * 128],
                rhs=qcols_sb[:, t0 + tt : t0 + tt + 1],
                start=True,
                stop=True,
            )
        return scores

    def emit_rest(t0, ch, scores):
        # attn = exp(s) straight from PSUM in bf16 (s <= ~83 < ln(bf16 max));
        # masking is free: hosts sets ids=-1 at invalid positions, so both
        # one-hots (incl. the is_ge Z column) are all-zero there
        attn = wkpool.tile([128, CHMAX], BF16, tag="attn")
        nc.scalar.activation(
            out=attn[:, :ch], in_=scores[:, :ch], func=AF.Exp, bias=zcol[:, 0:1], scale=1.0
        )
        oh_t = ohpool.tile([128, LO * CHMAX], BF16, tag="oh")
        oh_v = oh_t[:].rearrange("p (l t) -> p l t", t=CHMAX)
        nc.vector.tensor_tensor(
            out=oh_v[:, :, 0:ch],
            in0=idlo[:, t0 : t0 + ch]
            .rearrange("p (o t) -> p o t", o=1)
            .to_broadcast([128, LO, ch]),
            in1=iota33.rearrange("p (l t) -> p l t", t=CHMAX)[:, :, 0:ch],
            op=ALU.is_equal,
        )
        nc.vector.tensor_scalar(
            out=oh_t[:, 32 * CHMAX : 32 * CHMAX + ch],
            in0=idlo[:, t0 : t0 + ch],
            scalar1=0,
            scalar2=None,
            op0=ALU.is_ge,
        )
        w19 = ohpool.tile([128, HI * CHMAX], BF16, tag="w19")
        w19_v = w19[:].rearrange("p (h t) -> p h t", t=CHMAX)
        nc.vector.tensor_tensor(
            out=w19_v[:, :, 0:ch],
            in0=idhi[:, t0 : t0 + ch]
            .rearrange("p (o t) -> p o t", o=1)
            .to_broadcast([128, HI, ch]),
            in1=iota19.rearrange("p (h t) -> p h t", t=CHMAX)[:, :, 0:ch],
            op=ALU.is_equal,
        )
        w19a = ohpool.tile([128, HI * CHMAX], BF16, tag="w19a")
        w19a_v = w19a[:].rearrange("p (h t) -> p h t", t=CHMAX)
        nc.vector.tensor_tensor(
            out=w19a_v[:, :, 0:ch],
            in0=w19_v[:, :, 0:ch],
            in1=attn[:, :ch]
            .rearrange("p (o t) -> p o t", o=1)
            .to_broadcast([128, HI, ch]),
            op=ALU.mult,
        )
        for tt in range(ch):
            t = t0 + tt
            g = t // 2
            if g < Gb:
                bk = g // GPB
                if bk not in u_tiles:
                    u_tiles[bk] = upool.tile(
                        [HI, GPB * LO], F32, tag="u", name=f"u{bk}"
                    )
                o = u_tiles[bk][:, (g % GPB) * LO : (g % GPB) * LO + LO]
                st = t % 2 == 0
                sp = t % 2 == 1
            else:
                if "uF" not in u_tiles:
                    u_tiles["uF"] = upool.tile([HI, LO], F32, tag="u", name="uF")
                o = u_tiles["uF"][:]
                st = t == 2 * Gb
                sp = t == NT - 1
            nc.tensor.matmul(
                out=o, lhsT=w19a_v[:, :, tt], rhs=oh_v[:, :, tt], start=st, stop=sp
            )
        tend = t0 + ch
        while state["banks_done"] < NBb and 2 * min(
            (state["banks_done"] + 1) * GPB, Gb
        ) <= tend:
            b = state["banks_done"]
            done = 2 * min((b + 1) * GPB, Gb)
            # the LAST bank's chain is the tail's critical path: schedule its
            # copy immediately and its round-trip one step later
            d = 0 if b == NBb - 1 else 16
            pending_copy.append((done + d, b))
            state["banks_done"] += 1
        while pending_copy and pending_copy[0][0] <= tend:
            due, b = pending_copy.pop(0)
            pending_rt.append((due + 16, b, finalize_bank_copy(b)))
        while pending_rt and pending_rt[0][0] <= tend:
            due, b, usb_t = pending_rt.pop(0)
            pending_mms.append((due + 24, finalize_bank_rt(b, usb_t)))
        while pending_mms and pending_mms[0][0] <= tend:
            pending_mms.pop(0)[1]()

    # ---- main stream (software-pipelined emission) ----
    t0 = 0
    prev = None
    pending_copy = []
    pending_rt = []
    pending_mms = []
    for ch in sizes:
        scores = emit_front(t0, ch)
        if prev is not None:
            emit_rest(*prev)
        prev = (t0, ch, scores)
        t0 += ch
    emit_rest(*prev)
    assert state["banks_done"] == NBb, (state["banks_done"], NBb)
    # leftover bank round-trips (their group data is long since ready)
    while pending_copy:
        _, b = pending_copy.pop(0)
        pending_rt.append((0, b, finalize_bank_copy(b)))
    while pending_rt:
        _, b, usb_t = pending_rt.pop(0)
        pending_mms.append((0, finalize_bank_rt(b, usb_t)))
    if F > 0:
        finalize_fast()
    # leftover Sel matmuls go AFTER the fast-path matmuls: their xp read may
    # still be in flight and a parked matmul head-blocks the PE queue
    while pending_mms:
        pending_mms.pop(0)[1]()

    if NBb == 0:
        # no bank matmuls: close the accumulation groups with zero matmuls
        # (the uF identity matmuls only touch sub-regions of row 0)
        nc.tensor.matmul(
            out=A_ps[:], lhsT=zrow8[:], rhs=zwide[:, 0 : 13 * LO], start=False, stop=True
        )
        nc.tensor.matmul(
            out=B_ps[:], lhsT=zrow8[:], rhs=zwide[:, 0 : 6 * LO + 1], start=False, stop=True
        )

    # ---- finalize: invZ, then logits = Ln(u*invZ + eps) straight from PSUM ----
    zz = sb.tile([BL, 1], F32)
    nc.vector.reciprocal(out=zz[:], in_=B_ps[:, 6 * LO : 6 * LO + 1])
    lg = sb.tile([BL, OUTE], F32)
    nc.scalar.activation(
        out=lg[:, 0:416].rearrange("j (h l) -> j h l", h=13),
        in_=A_ps[:].rearrange("j (h l) -> j h l", h=13)[:, :, 0:32],
        func=AF.Ln,
        bias=epscol[:, 0:1],
        scale=zz[:, 0:1],
    )
    nc.scalar.activation(
        out=lg[:, 416:512].rearrange("j (h l) -> j h l", h=3),
        in_=B_ps[:, 0 : 6 * LO].rearrange("j (h l) -> j h l", h=6)[:, 0:3, 0:32],
        func=AF.Ln,
        bias=epscol[:, 0:1],
        scale=zz[:, 0:1],
    )
    nc.sync.dma_start(out=out, in_=lg[:])


def build_program(NT, F):
    Gn = NT // 2
    Gb = Gn - F
    NBb = (Gb + GPB - 1) // GPB
    nc = bacc.Bacc(
        "TRN2",
        target_bir_lowering=False,
        debug=False,
        enable_asserts=False,
        num_devices=1,
    )
    docp = nc.dram_tensor(
        "docp", [128, NT + NT * 128], F16, kind="ExternalInput"
    ).ap()  # cols [0:NT] = per-tile q values; [NT:] = packed docT
    ids_d = nc.dram_tensor("ids", [128, 2 * NT], I16, kind="ExternalInput").ap()
    selb_d = nc.dram_tensor(
        "selb", [GPB, max(NBb, 1) * BL], BF16, kind="ExternalInput"
    ).ap()
    eye_d = nc.dram_tensor("eye19", [HI, HI], BF16, kind="ExternalInput").ap()
    xdram = nc.dram_tensor(
        "xdram", [max(Gb, 1), HI * LO], BF16, kind="ExternalInput"
    ).ap()
    out = nc.dram_tensor("out", [BL, OUTE], F32, kind="ExternalOutput").ap()

    tensors = (out, docp, ids_d, selb_d, eye_d, xdram)
    with tile.TileContext(nc) as tc:
        with ExitStack() as ctx:
            emit_kernel(ctx, tc, NT, F, tensors)
    nc.compile()
    return nc


def make_in_maps(doc_emb, query_emb, doc_ids, seq_length, plan):
    NT = plan["NT"]
    F = plan["F"]
    Gn = NT // 2
    Gb = Gn - F
    NBb = (Gb + GPB - 1) // GPB
    gt = plan["gt"]
    L = plan["L"]
    eye = np.eye(HI, dtype=np.float32).astype(BF16NP)
    in_maps = []
    for c in range(NCORES):
        bs = plan["assign"][c]
        docq = np.zeros((128, NT + NT * 128), np.float16)
        docT = docq[:, NT:].reshape(128, NT, 128)
        qcols = docq[:, :NT]
        ids2 = np.full((128, 2 * NT), -1, np.int16)
        idlo = ids2[:, :NT]
        idhi = ids2[:, NT:]
        selb = np.zeros((GPB, max(NBb, 1) * BL), BF16NP)
        t0 = 0
        p = np.arange(128)
        for j, b in enumerate(bs):
            nt = int(gt[b])
            lj = int(L[b])
            npos = min(nt * 128, S)
            seg = np.zeros((nt * 128, E), np.float32)
            seg[:npos] = doc_emb[b, :npos, :]
            docT[:, t0 : t0 + nt, :] = (
                seg.reshape(nt, 128, E).transpose(2, 0, 1).astype(np.float16)
            )
            qcols[:, t0 : t0 + nt] = query_emb[b].astype(np.float16)[:, None]
            svals = (np.arange(nt) * 128)[None, :] + p[:, None]
            valid = svals < lj
            idseg = np.zeros(nt * 128, np.int32)
            idseg[:npos] = doc_ids[b, :npos]
            idseg = idseg.reshape(nt, 128).T
            idlo[:, t0 : t0 + nt] = np.where(valid, idseg & 31, -1).astype(np.int16)
            idhi[:, t0 : t0 + nt] = np.where(valid, idseg >> 5, -1).astype(np.int16)
            slot = (j + 1) % BL  # stream-last batch -> slot 0 (fast path)
            for g in range(t0 // 2, (t0 + nt) // 2):
                if g < Gb:
                    selb[g % GPB, (g // GPB) * BL + slot] = 1.0
                else:
                    # fast groups are added straight into slot 0
                    assert slot == 0, (c, j, g, Gb)
            t0 += nt
        in_maps.append(
            {
                "docp": docq,
                "ids": ids2,
                "selb": selb,
                "eye19": eye,
                "xdram": np.zeros((max(Gb, 1), HI * LO), BF16NP),
            }
        )
    return in_maps


_CACHE = {}


def _get_program(key=None):
    if key is None:
        key = _CACHE.get("last_key")
        assert key is not None, "no program built yet"
    if key not in _CACHE:
        _CACHE[key] = build_program(*key)
    _CACHE["last_key"] = key
    return _CACHE[key]


def kernel(**inputs):
    doc_emb = np.asarray(inputs["doc_emb"], dtype=np.float32)
    query_emb = np.asarray(inputs["query_emb"], dtype=np.float32)
    doc_ids = np.asarray(inputs["doc_ids"], dtype=np.int32)
    seq_length = np.asarray(inputs["seq_length"], dtype=np.int32)

    plan = make_plan(seq_length)
    nc = _get_program((plan["NT"], plan["F"]))
    in_maps = make_in_maps(doc_emb, query_emb, doc_ids, seq_length, plan)
    res = bass_utils.run_bass_kernel_spmd(nc, in_maps, core_ids=list(range(NCORES)))
    out = np.zeros((B, OUTE), np.float32)
    for c in range(NCORES):
        o = np.asarray(res.results[c]["out"], dtype=np.float32)
        for j, b in enumerate(plan["assign"][c]):
            out[b] = o[(j + 1) % BL]
    return out


# revision 52
# speedup vs baseline: 1.0437x; 1.0012x over previous
"""Trainium2 Bass kernel for nn_AttentionSumReader (segment_reduce).

Pipeline per batch (B=64, S=4096, E=128, 600 entities -> logits over first 512):
  scores = doc_emb @ query          (per-batch matvec)
  attn   = masked softmax(scores)   (mask: s < max(seq_length,1))
  sums   = segment_sum(attn, doc_ids)[:512]
  out    = log(sums + 1e-9)

Strategy (v3 — JIT length-specialized flat tile stream + fast-path tail):
  - Data-parallel over batch: 8 batches/core, LOAD-BALANCED across cores by
    valid length (seq_length known on host pre-compile); only the valid
    prefix of each batch is streamed. Host pre-transposes doc to [E, s]
    f16 and packs per-core tile streams (tile = 128 positions). Programs
    are compiled per realized (NT, F) and cached. ~2x traffic from
    f32->f16, ~1.7x from skipping invalid positions, no on-chip transpose.
  - Per-tile batch context is data, not control flow: host sends per-tile
    query columns, additive masks, id hi/lo (int16), so one SPMD program
    serves per-core variable batch boundaries.
  - Matvec: docT tile stationary, per-tile q column moving; scores [128,ch]
    in PSUM. attn = exp(s) via exp(s/4)^4 (ACT), -2000 additive mask;
    normalization deferred (logits need u/Z once at the end).
  - Segment-sum: id = hi*32+lo (hi<19, lo<32). One-hots on DVE in 2-byte
    2x mode (l-major layout, int16 ids vs int16 iota -> bf16). lo-one-hot
    col 32 is constant 1 so each u block's col 32 accumulates per-hi attn
    sums (gives Z). Per-tile matmul (lhsT=w_hi*attn [128,19],
    rhs=oh_lo [128,33]) accumulates per-GROUP u[19,33] in PSUM (group=2
    tiles); batches own whole groups.
  - Group->batch reduction, two paths into per-batch PSUM A[8,429]/B[8,198]
    (= [8, hi, 33] split at hi 13):
    * banked groups (all but last F): u banks [19, 15*33] -> bf16 (ACT) ->
      DRAM -> reload with groups on partitions -> Sel[g, slot] matmul
      (Sel host-built per core). Round-trips overlap the stream (SWDGE
      queue, keeps HWDGE free for the doc stream).
    * fast groups (last F, host guarantees they belong to slot 7 = the
      longest batch, or are zero-mass dummies): u[19,33] -> bf16 -> 19
      identity-column matmuls add rows straight into A/B[7] - no DRAM
      round-trip on the critical tail.
  - Finalize: Z_j = sum_hi A/B[j, hi*33+32]; logits = Ln(u*invZ + eps).
"""

import sys

sys.path.insert(0, "/opt/trn_rl_repo")

from contextlib import ExitStack

import numpy as np
import ml_dtypes

from concourse import bacc, bass, mybir, tile
from concourse import bass_utils

BF16NP = ml_dtypes.bfloat16

# ---- problem constants (hardcoded; kernel.py must be self-contained) ----
B, S, E = 64, 4096, 128
NCORES = 8
BL = B // NCORES  # batches per core
LO = 33  # 32 lo values + 1 ones-column (for Z)
HI = 19  # 600 entities <= 19*32
GPB = 15  # groups per PSUM bank: 15*33*4B = 1980 <= 2048
CHMAX = 16  # max tiles per processing chunk
FMAX = 64  # max fast-path (no-roundtrip) tail groups
OUTE = 512
EPS = 1e-9

F32 = mybir.dt.float32
F16 = mybir.dt.float16
BF16 = mybir.dt.bfloat16
I16 = mybir.dt.int16

ALU = mybir.AluOpType
AF = mybir.ActivationFunctionType
AX = mybir.AxisListType


def make_plan(seq_length):
    """Balance batches across cores by padded valid-tile count; derive the
    uniform per-core stream length NT, fast-group count F, chunk split."""
    L = np.maximum(np.asarray(seq_length, dtype=np.int64), 1)
    tiles = (L + 127) // 128
    gt = 2 * ((tiles + 1) // 2)  # pad each batch to whole groups (G=2)
    order = np.argsort(-gt, kind="stable")
    loads = [0] * NCORES
    counts = [0] * NCORES
    assign = [[] for _ in range(NCORES)]
    for b in order:
        c = min(
            (i for i in range(NCORES) if counts[i] < BL), key=lambda i: loads[i]
        )
        loads[c] += int(gt[b])
        counts[c] += 1
        assign[c].append(int(b))
    # slot order ascending by length: slot BL-1 (stream-last) = longest batch
    assign = [list(reversed(a)) for a in assign]
    NT = int(max(loads))
    NT = max(NT, 4)
    if NT % 2:
        NT += 1
    Gn = NT // 2
    # fast groups must contain only slot-7 tiles or dummies on EVERY core
    start_g7 = []
    for c in range(NCORES):
        pre = sum(int(gt[b]) for b in assign[c][: BL - 1])
        start_g7.append(pre // 2)
    F = max(0, min(FMAX, Gn - max(start_g7)))
    return {"assign": assign, "gt": gt, "L": L, "NT": NT, "F": F}


def make_chunk_sizes(NT, Gb):
    """Even-sized chunks covering [0, 2*Gb) then [2*Gb, NT): a chunk boundary
    lands exactly on the banked/fast split so the last bank's DRAM round-trip
    starts as early as possible. Small first chunk for a fast pipeline head."""

    def split(n, first=None):
        sizes = []
        if first and n >= first:
            sizes.append(first)
            n -= first
        while n > CHMAX:
            sizes.append(CHMAX)
            n -= CHMAX
        if n:
            sizes.append(n)
        return sizes

    return split(2 * Gb, first=4), split(NT - 2 * Gb)


def emit_kernel(ctx, tc, NT, F, tensors):
    nc = tc.nc
    Gn = NT // 2
    Gb = Gn - F  # banked groups
    NBb = (Gb + GPB - 1) // GPB
    sizes_a, sizes_b = make_chunk_sizes(NT, Gb)
    sizes = sizes_a + sizes_b

    (out, docp, ids_d, selb_d, eye_d, xdram) = tensors

    sb = ctx.enter_context(tc.tile_pool(name="sb", bufs=1))
    docpool = ctx.enter_context(tc.tile_pool(name="docp", bufs=8))
    wkpool = ctx.enter_context(tc.tile_pool(name="wk", bufs=4))
    ohpool = ctx.enter_context(tc.tile_pool(name="oh", bufs=4))
    usbpool = ctx.enter_context(tc.tile_pool(name="usb", bufs=2))
    ufpool = ctx.enter_context(tc.tile_pool(name="ufp", bufs=1))
    xppool = ctx.enter_context(tc.tile_pool(name="xp", bufs=max(NBb, 1)))
    scpool = ctx.enter_context(tc.tile_pool(name="sc", bufs=3, space="PSUM"))
    upool = ctx.enter_context(tc.tile_pool(name="up", bufs=3, space="PSUM"))
    abpool = ctx.enter_context(tc.tile_pool(name="ab", bufs=1, space="PSUM"))

    # ---- small inputs ----
    iota33_t = sb.tile([128, LO * CHMAX], I16)
    nc.gpsimd.iota(
        iota33_t[:], pattern=[[1, LO], [0, CHMAX]], base=0, channel_multiplier=0
    )
    iota19_t = sb.tile([128, HI * CHMAX], I16)
    nc.gpsimd.iota(
        iota19_t[:], pattern=[[1, HI], [0, CHMAX]], base=0, channel_multiplier=0
    )
    iota33 = iota33_t[:]
    iota19 = iota19_t[:]
    # ACT/HWDGE queue: ids in one small DMA (transfers are tiny)
    ids_t = sb.tile([128, 2 * NT], I16)
    nc.scalar.dma_start(out=ids_t[:], in_=ids_d)
    idlo = ids_t[:, 0:NT]
    idhi = ids_t[:, NT : 2 * NT]
    selb = sb.tile([GPB, max(NBb, 1) * BL], BF16)
    if NBb:
        nc.gpsimd.dma_start(out=selb[:], in_=selb_d)
    eye19 = sb.tile([HI, HI], BF16)
    nc.gpsimd.dma_start(out=eye19[:], in_=eye_d)
    zcol = sb.tile([128, 1], F32)
    nc.vector.memset(zcol[:], 0.0)
    epscol = sb.tile([BL, 1], F32)
    nc.vector.memset(epscol[:], EPS)
    ones19 = sb.tile([HI, 1], BF16)
    nc.vector.memset(ones19[:], 1.0)
    zrow8 = sb.tile([1, BL], BF16)
    nc.vector.memset(zrow8[:], 0.0)
    zwide = sb.tile([1, 13 * LO], BF16)
    nc.vector.memset(zwide[:], 0.0)
    # pin the combined exp+ln activation table once, up front: the auto
    # table-load pass then inserts nothing (no 1.3us reload before the final
    # Ln on the critical tail)
    from concourse.hw_specs import get_activation_tables

    tabs = list(get_activation_tables(nc.m.arch).items())
    combined = next(
        i for i, (k, v) in enumerate(tabs) if AF.Exp in v and AF.Ln in v
    )
    nc.scalar.add_instruction(
        mybir.InstLoadActFuncSet(
            name=nc.get_next_instruction_name(),
            act_func_set_id=combined,
            ins=[],
            outs=[],
        )
    )

    u_tiles = {}
    A_ps = abpool.tile([BL, 13 * LO], F32, tag="A")
    B_ps = abpool.tile([BL, 6 * LO + 1], F32, tag="B")  # col 198 = Z
    # A/B accumulation bookkeeping: first writer starts, closers stop.
    a_seen = [0]
    b_seen = [0]

    def finalize_bank_copy(b):
        gl = min(GPB, Gb - b * GPB)
        usb_t = usbpool.tile([HI, GPB * LO], BF16, tag="usb")
        nc.vector.tensor_copy(out=usb_t[:, : gl * LO], in_=u_tiles[b][:, : gl * LO])
        return usb_t

    def finalize_bank_rt(b, usb_t):
        gl = min(GPB, Gb - b * GPB)
        # write on the SP queue (its wait is long satisfied; the doc queue has
        # ~800ns/chunk of issue slack); read alone on the Pool queue, where
        # its ~2.3us wait on the write's completion blocks nothing else
        nc.sync.dma_start(
            out=xdram[b * GPB : b * GPB + gl, :].rearrange("g (h l) -> h g l", h=HI),
            in_=usb_t[:, : gl * LO].rearrange("h (g l) -> h g l", g=gl),
        )
        xp_t = xppool.tile([GPB, HI * LO + 1], BF16, tag="xp")
        nc.gpsimd.dma_start(
            out=xp_t[:gl, 0 : HI * LO], in_=xdram[b * GPB : b * GPB + gl, :]
        )

        gl = min(GPB, Gb - b * GPB)

        def mms():
            # per-group total attn (for Z): sum the 19 col-32 entries per row
            with nc.allow_low_precision(reason="Z column; rel err ~2^-8 on ln(Z)"):
                nc.vector.tensor_reduce(
                    out=xp_t[:gl, HI * LO : HI * LO + 1],
                    in_=xp_t[:gl, 0 : HI * LO].rearrange("g (h l) -> g h l", l=LO)[
                        :, :, 32
                    ],
                    axis=AX.X,
                    op=ALU.add,
                )
            a_seen[0] += 1
            # the last bank's Sel matmuls are emitted last among all A/B
            # writers and cover the FULL regions, so they close the psum
            # accumulation groups (no separate zero-closers needed)
            nc.tensor.matmul(
                out=A_ps[:],
                lhsT=selb[0:gl, b * BL : (b + 1) * BL],
                rhs=xp_t[0:gl, 0 : 13 * LO],
                start=(a_seen[0] == 1),
                stop=(b == NBb - 1),
            )
            b_seen[0] += 1
            nc.tensor.matmul(
                out=B_ps[:],
                lhsT=selb[0:gl, b * BL : (b + 1) * BL],
                rhs=xp_t[0:gl, 13 * LO : HI * LO + 1],
                start=(b_seen[0] == 1),
                stop=(b == NBb - 1),
            )

        return mms

    def finalize_fast():
        # the whole fast region belongs to the stream-last batch, which the
        # host maps to SLOT 0 (PE matmul out base partition must be 0/32/64);
        # it was accumulated into the single uF psum tile. One bf16 copy, then
        # 19 identity-column matmuls scatter its rows into A/B[0], plus one
        # ones-column matmul for its Z contribution.
        uf_sb = ufpool.tile([HI, LO], BF16, tag="uf")
        nc.vector.tensor_copy(out=uf_sb[:], in_=u_tiles["uF"][:])
        a_seen[0] += 1
        b_seen[0] += 1
        for h in range(HI):
            if h < 13:
                o = A_ps[0:1, h * LO : (h + 1) * LO]
                st = a_seen[0] == 1 and h == 0
            else:
                o = B_ps[0:1, (h - 13) * LO : (h - 12) * LO]
                st = b_seen[0] == 1 and h == 13
            nc.tensor.matmul(
                out=o, lhsT=eye19[:, h : h + 1], rhs=uf_sb[:], start=st, stop=False
            )
        nc.tensor.matmul(
            out=B_ps[0:1, 6 * LO : 6 * LO + 1],
            lhsT=ones19[:],
            rhs=uf_sb[:, 32:33],
            start=False,
            stop=False,
        )

    state = {"banks_done": 0}
    qcols_sb = sb.tile([128, NT], F16)

    def emit_front(t0, ch):
        """doc DMA + matvecs: emitted a chunk ahead so the PE queue never
        head-blocks the next chunk's matvecs behind this chunk's segmats."""
        doc_t = docpool.tile([128, CHMAX * 128], F16, tag="doc")
        nc.sync.dma_start(
            out=doc_t[:, : ch * 128],
            in_=docp[:, NT + t0 * 128 : NT + (t0 + ch) * 128],
        )
        if t0 == 0:
            nc.scalar.dma_start(out=qcols_sb[:], in_=docp[:, 0:NT])
        scores = scpool.tile([128, CHMAX], F32, tag="sc")
        for tt in range(ch):
            nc.tensor.matmul(
                out=scores[:, tt : tt + 1],
                lhsT=doc_t[:, tt * 128 : (tt + 1) * 128],
                rhs=qcols_sb[:, t0 + tt : t0 + tt + 1],
                start=True,
                stop=True,
            )
        return scores

    def emit_rest(t0, ch, scores):
        # attn = exp(s) straight from PSUM in bf16 (s <= ~83 < ln(bf16 max));
        # masking is free: hosts sets ids=-1 at invalid positions, so both
        # one-hots (incl. the is_ge Z column) are all-zero there
        attn = wkpool.tile([128, CHMAX], BF16, tag="attn")
        nc.scalar.activation(
            out=attn[:, :ch], in_=scores[:, :ch], func=AF.Exp, bias=zcol[:, 0:1], scale=1.0
        )
        oh_t = ohpool.tile([128, LO * CHMAX], BF16, tag="oh")
        oh_v = oh_t[:].rearrange("p (l t) -> p l t", t=CHMAX)
        nc.vector.tensor_tensor(
            out=oh_v[:, :, 0:ch],
            in0=idlo[:, t0 : t0 + ch]
            .rearrange("p (o t) -> p o t", o=1)
            .to_broadcast([128, LO, ch]),
            in1=iota33.rearrange("p (l t) -> p l t", t=CHMAX)[:, :, 0:ch],
            op=ALU.is_equal,
        )
        nc.vector.tensor_scalar(
            out=oh_t[:, 32 * CHMAX : 32 * CHMAX + ch],
            in0=idlo[:, t0 : t0 + ch],
            scalar1=0,
            scalar2=None,
            op0=ALU.is_ge,
        )
        w19 = ohpool.tile([128, HI * CHMAX], BF16, tag="w19")
        w19_v = w19[:].rearrange("p (h t) -> p h t", t=CHMAX)
        nc.vector.tensor_tensor(
            out=w19_v[:, :, 0:ch],
            in0=idhi[:, t0 : t0 + ch]
            .rearrange("p (o t) -> p o t", o=1)
            .to_broadcast([128, HI, ch]),
            in1=iota19.rearrange("p (h t) -> p h t", t=CHMAX)[:, :, 0:ch],
            op=ALU.is_equal,
        )
        w19a = ohpool.tile([128, HI * CHMAX], BF16, tag="w19a")
        w19a_v = w19a[:].rearrange("p (h t) -> p h t", t=CHMAX)
        nc.vector.tensor_tensor(
            out=w19a_v[:, :, 0:ch],
            in0=w19_v[:, :, 0:ch],
            in1=attn[:, :ch]
            .rearrange("p (o t) -> p o t", o=1)
            .to_broadcast([128, HI, ch]),
            op=ALU.mult,
        )
        for tt in range(ch):
            t = t0 + tt
            g = t // 2
            if g < Gb:
                bk = g // GPB
                if bk not in u_tiles:
                    u_tiles[bk] = upool.tile(
                        [HI, GPB * LO], F32, tag="u", name=f"u{bk}"
                    )
                o = u_tiles[bk][:, (g % GPB) * LO : (g % GPB) * LO + LO]
                st = t % 2 == 0
                sp = t % 2 == 1
            else:
                if "uF" not in u_tiles:
                    u_tiles["uF"] = upool.tile([HI, LO], F32, tag="u", name="uF")
                o = u_tiles["uF"][:]
                st = t == 2 * Gb
                sp = t == NT - 1
            nc.tensor.matmul(
                out=o, lhsT=w19a_v[:, :, tt], rhs=oh_v[:, :, tt], start=st, stop=sp
            )
        tend = t0 + ch
        while state["banks_done"] < NBb and 2 * min(
            (state["banks_done"] + 1) * GPB, Gb
        ) <= tend:
            b = state["banks_done"]
            done = 2 * min((b + 1) * GPB, Gb)
            # the LAST bank's chain is the tail's critical path: schedule its
            # copy immediately and its round-trip one step later
            d = 0 if b == NBb - 1 else 16
            pending_copy.append((done + d, b))
            state["banks_done"] += 1
        while pending_copy and pending_copy[0][0] <= tend:
            due, b = pending_copy.pop(0)
            pending_rt.append((due + 16, b, finalize_bank_copy(b)))
        while pending_rt and pending_rt[0][0] <= tend:
            due, b, usb_t = pending_rt.pop(0)
            pending_mms.append((due + 24, finalize_bank_rt(b, usb_t)))
        while pending_mms and pending_mms[0][0] <= tend:
            pending_mms.pop(0)[1]()

    # ---- main stream (software-pipelined emission) ----
    t0 = 0
    prev = None
    pending_copy = []
    pending_rt = []
    pending_mms = []
    for ch in sizes:
        scores = emit_front(t0, ch)
        if prev is not None:
            emit_rest(*prev)
        prev = (t0, ch, scores)
        t0 += ch
    emit_rest(*prev)
    assert state["banks_done"] == NBb, (state["banks_done"], NBb)
    # leftover bank round-trips (their group data is long since ready)
    while pending_copy:
        _, b = pending_copy.pop(0)
        pending_rt.append((0, b, finalize_bank_copy(b)))
    while pending_rt:
        _, b, usb_t = pending_rt.pop(0)
        pending_mms.append((0, finalize_bank_rt(b, usb_t)))
    if F > 0:
        finalize_fast()
    # leftover Sel matmuls go AFTER the fast-path matmuls: their xp read may
    # still be in flight and a parked matmul head-blocks the PE queue
    while pending_mms:
        pending_mms.pop(0)[1]()

    if NBb == 0:
        # no bank matmuls: close the accumulation groups with zero matmuls
        # (the uF identity matmuls only touch sub-regions of row 0)
        nc.tensor.matmul(
            out=A_ps[:], lhsT=zrow8[:], rhs=zwide[:, 0 : 13 * LO], start=False, stop=True
        )
        nc.tensor.matmul(
            out=B_ps[:], lhsT=zrow8[:], rhs=zwide[:, 0 : 6 * LO + 1], start=False, stop=True
        )

    # ---- finalize: invZ, then logits = Ln(u*invZ + eps) straight from PSUM ----
    zz = sb.tile([BL, 1], F32)
    nc.vector.reciprocal(out=zz[:], in_=B_ps[:, 6 * LO : 6 * LO + 1])
    lg = sb.tile([BL, OUTE], F32)
    nc.scalar.activation(
        out=lg[:, 0:416].rearrange("j (h l) -> j h l", h=13),
        in_=A_ps[:].rearrange("j (h l) -> j h l", h=13)[:, :, 0:32],
        func=AF.Ln,
        bias=epscol[:, 0:1],
        scale=zz[:, 0:1],
    )
    nc.scalar.activation(
        out=lg[:, 416:512].rearrange("j (h l) -> j h l", h=3),
        in_=B_ps[:, 0 : 6 * LO].rearrange("j (h l) -> j h l", h=6)[:, 0:3, 0:32],
        func=AF.Ln,
        bias=epscol[:, 0:1],
        scale=zz[:, 0:1],
    )
    nc.sync.dma_start(out=out, in_=lg[:])


def build_program(NT, F):
    Gn = NT // 2
    Gb = Gn - F
    NBb = (Gb + GPB - 1) // GPB
    nc = bacc.Bacc(
        "TRN2",
        target_bir_lowering=False,
        debug=False,
        enable_asserts=False,
        num_devices=1,
    )
    docp = nc.dram_tensor(
        "docp", [128, NT + NT * 128], F16, kind="ExternalInput"
    ).ap()  # cols [0:NT] = per-tile q values; [NT:] = packed docT
    ids_d = nc.dram_tensor("ids", [128, 2 * NT], I16, kind="ExternalInput").ap()
    selb_d = nc.dram_tensor(
        "selb", [GPB, max(NBb, 1) * BL], BF16, kind="ExternalInput"
    ).ap()
    eye_d = nc.dram_tensor("eye19", [HI, HI], BF16, kind="ExternalInput").ap()
    xdram = nc.dram_tensor(
        "xdram", [max(Gb, 1), HI * LO], BF16, kind="ExternalInput"
    ).ap()
    out = nc.dram_tensor("out", [BL, OUTE], F32, kind="ExternalOutput").ap()

    tensors = (out, docp, ids_d, selb_d, eye_d, xdram)
    with tile.TileContext(nc) as tc:
        with ExitStack() as ctx:
            emit_kernel(ctx, tc, NT, F, tensors)
    nc.compile()
    return nc


def make_in_maps(doc_emb, query_emb, doc_ids, seq_length, plan):
    NT = plan["NT"]
    F = plan["F"]
    Gn = NT // 2
    Gb = Gn - F
    NBb = (Gb + GPB - 1) // GPB
    gt = plan["gt"]
    L = plan["L"]
    eye = np.eye(HI, dtype=np.float32).astype(BF16NP)
    in_maps = []
    for c in range(NCORES):
        bs = plan["assign"][c]
        docq = np.zeros((128, NT + NT * 128), np.float16)
        docT = docq[:, NT:].reshape(128, NT, 128)
        qcols = docq[:, :NT]
        ids2 = np.full((128, 2 * NT), -1, np.int16)
        idlo = ids2[:, :NT]
        idhi = ids2[:, NT:]
        selb = np.zeros((GPB, max(NBb, 1) * BL), BF16NP)
        t0 = 0
        p = np.arange(128)
        for j, b in enumerate(bs):
            nt = int(gt[b])
            lj = int(L[b])
            npos = min(nt * 128, S)
            seg = np.zeros((nt * 128, E), np.float32)
            seg[:npos] = doc_emb[b, :npos, :]
            docT[:, t0 : t0 + nt, :] = (
                seg.reshape(nt, 128, E).transpose(2, 0, 1).astype(np.float16)
            )
            qcols[:, t0 : t0 + nt] = query_emb[b].astype(np.float16)[:, None]
            svals = (np.arange(nt) * 128)[None, :] + p[:, None]
            valid = svals < lj
            idseg = np.zeros(nt * 128, np.int32)
            idseg[:npos] = doc_ids[b, :npos]
            idseg = idseg.reshape(nt, 128).T
            idlo[:, t0 : t0 + nt] = np.where(valid, idseg & 31, -1).astype(np.int16)
            idhi[:, t0 : t0 + nt] = np.where(valid, idseg >> 5, -1).astype(np.int16)
            slot = (j + 1) % BL  # stream-last batch -> slot 0 (fast path)
            for g in range(t0 // 2, (t0 + nt) // 2):
                if g < Gb:
                    selb[g % GPB, (g // GPB) * BL + slot] = 1.0
                else:
                    # fast groups are added straight into slot 0
                    assert slot == 0, (c, j, g, Gb)
            t0 += nt
        in_maps.append(
            {
                "docp": docq,
                "ids": ids2,
                "selb": selb,
                "eye19": eye,
                "xdram": np.zeros((max(Gb, 1), HI * LO), BF16NP),
            }
        )
    return in_maps


_CACHE = {}


def _get_program(key=None):
    if key is None:
        key = _CACHE.get("last_key")
        assert key is not None, "no program built yet"
    if key not in _CACHE:
        _CACHE[key] = build_program(*key)
    _CACHE["last_key"] = key
    return _CACHE[key]


def kernel(**inputs):
    doc_emb = np.asarray(inputs["doc_emb"], dtype=np.float32)
    query_emb = np.asarray(inputs["query_emb"], dtype=np.float32)
    doc_ids = np.asarray(inputs["doc_ids"], dtype=np.int32)
    seq_length = np.asarray(inputs["seq_length"], dtype=np.int32)

    plan = make_plan(seq_length)
    nc = _get_program((plan["NT"], plan["F"]))
    in_maps = make_in_maps(doc_emb, query_emb, doc_ids, seq_length, plan)
    res = bass_utils.run_bass_kernel_spmd(nc, in_maps, core_ids=list(range(NCORES)))
    out = np.zeros((B, OUTE), np.float32)
    for c in range(NCORES):
        o = np.asarray(res.results[c]["out"], dtype=np.float32)
        for j, b in enumerate(plan["assign"][c]):
            out[b] = o[(j + 1) % BL]
    return out


# revision 53
# speedup vs baseline: 1.0781x; 1.0329x over previous
"""Trainium2 Bass kernel for nn_AttentionSumReader (segment_reduce).

Pipeline per batch (B=64, S=4096, E=128, 600 entities -> logits over first 512):
  scores = doc_emb @ query          (per-batch matvec)
  attn   = masked softmax(scores)   (mask: s < max(seq_length,1))
  sums   = segment_sum(attn, doc_ids)[:512]
  out    = log(sums + 1e-9)

Strategy (v3 — JIT length-specialized flat tile stream + fast-path tail):
  - Data-parallel over batch: 8 batches/core, LOAD-BALANCED across cores by
    valid length (seq_length known on host pre-compile); only the valid
    prefix of each batch is streamed. Host pre-transposes doc to [E, s]
    f16 and packs per-core tile streams (tile = 128 positions). Programs
    are compiled per realized (NT, F) and cached. ~2x traffic from
    f32->f16, ~1.7x from skipping invalid positions, no on-chip transpose.
  - Per-tile batch context is data, not control flow: host sends per-tile
    query columns, additive masks, id hi/lo (int16), so one SPMD program
    serves per-core variable batch boundaries.
  - Matvec: docT tile stationary, per-tile q column moving; scores [128,ch]
    in PSUM. attn = exp(s) via exp(s/4)^4 (ACT), -2000 additive mask;
    normalization deferred (logits need u/Z once at the end).
  - Segment-sum: id = hi*32+lo (hi<19, lo<32). One-hots on DVE in 2-byte
    2x mode (l-major layout, int16 ids vs int16 iota -> bf16). lo-one-hot
    col 32 is constant 1 so each u block's col 32 accumulates per-hi attn
    sums (gives Z). Per-tile matmul (lhsT=w_hi*attn [128,19],
    rhs=oh_lo [128,33]) accumulates per-GROUP u[19,33] in PSUM (group=2
    tiles); batches own whole groups.
  - Group->batch reduction, two paths into per-batch PSUM A[8,429]/B[8,198]
    (= [8, hi, 33] split at hi 13):
    * banked groups (all but last F): u banks [19, 15*33] -> bf16 (ACT) ->
      DRAM -> reload with groups on partitions -> Sel[g, slot] matmul
      (Sel host-built per core). Round-trips overlap the stream (SWDGE
      queue, keeps HWDGE free for the doc stream).
    * fast groups (last F, host guarantees they belong to slot 7 = the
      longest batch, or are zero-mass dummies): u[19,33] -> bf16 -> 19
      identity-column matmuls add rows straight into A/B[7] - no DRAM
      round-trip on the critical tail.
  - Finalize: Z_j = sum_hi A/B[j, hi*33+32]; logits = Ln(u*invZ + eps).
"""

import sys

sys.path.insert(0, "/opt/trn_rl_repo")

from contextlib import ExitStack

import numpy as np
import ml_dtypes

from concourse import bacc, bass, mybir, tile
from concourse import bass_utils

BF16NP = ml_dtypes.bfloat16

# ---- problem constants (hardcoded; kernel.py must be self-contained) ----
B, S, E = 64, 4096, 128
NCORES = 8
BL = B // NCORES  # batches per core
LO = 33  # 32 lo values + 1 ones-column (for Z)
HI = 19  # 600 entities <= 19*32
GPB = 15  # groups per PSUM bank: 15*33*4B = 1980 <= 2048
CHMAX = 16  # max tiles per processing chunk
FMAX = 64  # max fast-path (no-roundtrip) tail groups
OUTE = 512
EPS = 1e-9

F32 = mybir.dt.float32
F16 = mybir.dt.float16
BF16 = mybir.dt.bfloat16
I16 = mybir.dt.int16

ALU = mybir.AluOpType
AF = mybir.ActivationFunctionType
AX = mybir.AxisListType


def make_plan(seq_length):
    """Balance batches across cores by padded valid-tile count; derive the
    uniform per-core stream length NT, fast-group count F, chunk split."""
    L = np.maximum(np.asarray(seq_length, dtype=np.int64), 1)
    tiles = (L + 127) // 128
    gt = 2 * ((tiles + 1) // 2)  # pad each batch to whole groups (G=2)
    order = np.argsort(-gt, kind="stable")
    loads = [0] * NCORES
    counts = [0] * NCORES
    assign = [[] for _ in range(NCORES)]
    for b in order:
        c = min(
            (i for i in range(NCORES) if counts[i] < BL), key=lambda i: loads[i]
        )
        loads[c] += int(gt[b])
        counts[c] += 1
        assign[c].append(int(b))
    # slot order ascending by length: slot BL-1 (stream-last) = longest batch
    assign = [list(reversed(a)) for a in assign]
    NT = int(max(loads))
    NT = max(NT, 4)
    if NT % 2:
        NT += 1
    Gn = NT // 2
    # fast groups must contain only slot-7 tiles or dummies on EVERY core
    start_g7 = []
    for c in range(NCORES):
        pre = sum(int(gt[b]) for b in assign[c][: BL - 1])
        start_g7.append(pre // 2)
    F = max(0, min(FMAX, Gn - max(start_g7)))
    return {"assign": assign, "gt": gt, "L": L, "NT": NT, "F": F}


def make_chunk_sizes(NT, Gb):
    """Even-sized chunks covering [0, 2*Gb) then [2*Gb, NT): a chunk boundary
    lands exactly on the banked/fast split so the last bank's DRAM round-trip
    starts as early as possible. Small first chunk for a fast pipeline head."""

    def split(n, first=None):
        sizes = []
        if first and n >= first:
            sizes.append(first)
            n -= first
        while n > CHMAX:
            sizes.append(CHMAX)
            n -= CHMAX
        if n:
            sizes.append(n)
        return sizes

    return split(2 * Gb, first=8), split(NT - 2 * Gb)


def emit_kernel(ctx, tc, NT, F, tensors):
    nc = tc.nc
    Gn = NT // 2
    Gb = Gn - F  # banked groups
    NBb = (Gb + GPB - 1) // GPB
    sizes_a, sizes_b = make_chunk_sizes(NT, Gb)
    sizes = sizes_a + sizes_b

    (out, docp, ids_d, selb_d, eye_d, xdram) = tensors

    sb = ctx.enter_context(tc.tile_pool(name="sb", bufs=1))
    docpool = ctx.enter_context(tc.tile_pool(name="docp", bufs=8))
    wkpool = ctx.enter_context(tc.tile_pool(name="wk", bufs=4))
    ohpool = ctx.enter_context(tc.tile_pool(name="oh", bufs=4))
    usbpool = ctx.enter_context(tc.tile_pool(name="usb", bufs=2))
    ufpool = ctx.enter_context(tc.tile_pool(name="ufp", bufs=1))
    xppool = ctx.enter_context(tc.tile_pool(name="xp", bufs=max(NBb, 1)))
    scpool = ctx.enter_context(tc.tile_pool(name="sc", bufs=3, space="PSUM"))
    upool = ctx.enter_context(tc.tile_pool(name="up", bufs=3, space="PSUM"))
    abpool = ctx.enter_context(tc.tile_pool(name="ab", bufs=1, space="PSUM"))

    # ---- small inputs ----
    iota33_t = sb.tile([128, LO * CHMAX], I16)
    nc.gpsimd.iota(
        iota33_t[:], pattern=[[1, LO], [0, CHMAX]], base=0, channel_multiplier=0
    )
    iota19_t = sb.tile([128, HI * CHMAX], I16)
    nc.gpsimd.iota(
        iota19_t[:], pattern=[[1, HI], [0, CHMAX]], base=0, channel_multiplier=0
    )
    iota33 = iota33_t[:]
    iota19 = iota19_t[:]
    # ACT/HWDGE queue: ids in one small DMA (transfers are tiny)
    ids_t = sb.tile([128, 2 * NT], I16)
    nc.scalar.dma_start(out=ids_t[:], in_=ids_d)
    idlo = ids_t[:, 0:NT]
    idhi = ids_t[:, NT : 2 * NT]
    selb = sb.tile([GPB, max(NBb, 1) * BL], BF16)
    if NBb:
        nc.gpsimd.dma_start(out=selb[:], in_=selb_d)
    eye19 = sb.tile([HI, HI], BF16)
    nc.gpsimd.dma_start(out=eye19[:], in_=eye_d)
    zcol = sb.tile([128, 1], F32)
    nc.vector.memset(zcol[:], 0.0)
    epscol = sb.tile([BL, 1], F32)
    nc.vector.memset(epscol[:], EPS)
    ones19 = sb.tile([HI, 1], BF16)
    nc.vector.memset(ones19[:], 1.0)
    zrow8 = sb.tile([1, BL], BF16)
    nc.vector.memset(zrow8[:], 0.0)
    zwide = sb.tile([1, 13 * LO], BF16)
    nc.vector.memset(zwide[:], 0.0)
    # pin the combined exp+ln activation table once, up front: the auto
    # table-load pass then inserts nothing (no 1.3us reload before the final
    # Ln on the critical tail)
    from concourse.hw_specs import get_activation_tables

    tabs = list(get_activation_tables(nc.m.arch).items())
    combined = next(
        i for i, (k, v) in enumerate(tabs) if AF.Exp in v and AF.Ln in v
    )
    nc.scalar.add_instruction(
        mybir.InstLoadActFuncSet(
            name=nc.get_next_instruction_name(),
            act_func_set_id=combined,
            ins=[],
            outs=[],
        )
    )

    u_tiles = {}
    A_ps = abpool.tile([BL, 13 * LO], F32, tag="A")
    B_ps = abpool.tile([BL, 6 * LO + 1], F32, tag="B")  # col 198 = Z
    # A/B accumulation bookkeeping: first writer starts, closers stop.
    a_seen = [0]
    b_seen = [0]

    def finalize_bank_copy(b):
        gl = min(GPB, Gb - b * GPB)
        usb_t = usbpool.tile([HI, GPB * LO], BF16, tag="usb")
        nc.vector.tensor_copy(out=usb_t[:, : gl * LO], in_=u_tiles[b][:, : gl * LO])
        return usb_t

    def finalize_bank_rt(b, usb_t):
        gl = min(GPB, Gb - b * GPB)
        # write on the SP queue (its wait is long satisfied; the doc queue has
        # ~800ns/chunk of issue slack); read alone on the Pool queue, where
        # its ~2.3us wait on the write's completion blocks nothing else
        nc.sync.dma_start(
            out=xdram[b * GPB : b * GPB + gl, :].rearrange("g (h l) -> h g l", h=HI),
            in_=usb_t[:, : gl * LO].rearrange("h (g l) -> h g l", g=gl),
        )
        xp_t = xppool.tile([GPB, HI * LO + 1], BF16, tag="xp")
        nc.gpsimd.dma_start(
            out=xp_t[:gl, 0 : HI * LO], in_=xdram[b * GPB : b * GPB + gl, :]
        )

        gl = min(GPB, Gb - b * GPB)

        def mms():
            # per-group total attn (for Z): sum the 19 col-32 entries per row
            with nc.allow_low_precision(reason="Z column; rel err ~2^-8 on ln(Z)"):
                nc.vector.tensor_reduce(
                    out=xp_t[:gl, HI * LO : HI * LO + 1],
                    in_=xp_t[:gl, 0 : HI * LO].rearrange("g (h l) -> g h l", l=LO)[
                        :, :, 32
                    ],
                    axis=AX.X,
                    op=ALU.add,
                )
            a_seen[0] += 1
            # the last bank's Sel matmuls are emitted last among all A/B
            # writers and cover the FULL regions, so they close the psum
            # accumulation groups (no separate zero-closers needed)
            nc.tensor.matmul(
                out=A_ps[:],
                lhsT=selb[0:gl, b * BL : (b + 1) * BL],
                rhs=xp_t[0:gl, 0 : 13 * LO],
                start=(a_seen[0] == 1),
                stop=(b == NBb - 1),
            )
            b_seen[0] += 1
            nc.tensor.matmul(
                out=B_ps[:],
                lhsT=selb[0:gl, b * BL : (b + 1) * BL],
                rhs=xp_t[0:gl, 13 * LO : HI * LO + 1],
                start=(b_seen[0] == 1),
                stop=(b == NBb - 1),
            )

        return mms

    def finalize_fast():
        # the whole fast region belongs to the stream-last batch, which the
        # host maps to SLOT 0 (PE matmul out base partition must be 0/32/64);
        # it was accumulated into the single uF psum tile. One bf16 copy, then
        # 19 identity-column matmuls scatter its rows into A/B[0], plus one
        # ones-column matmul for its Z contribution.
        uf_sb = ufpool.tile([HI, LO], BF16, tag="uf")
        nc.vector.tensor_copy(out=uf_sb[:], in_=u_tiles["uF"][:])
        a_seen[0] += 1
        b_seen[0] += 1
        for h in range(HI):
            if h < 13:
                o = A_ps[0:1, h * LO : (h + 1) * LO]
                st = a_seen[0] == 1 and h == 0
            else:
                o = B_ps[0:1, (h - 13) * LO : (h - 12) * LO]
                st = b_seen[0] == 1 and h == 13
            nc.tensor.matmul(
                out=o, lhsT=eye19[:, h : h + 1], rhs=uf_sb[:], start=st, stop=False
            )
        nc.tensor.matmul(
            out=B_ps[0:1, 6 * LO : 6 * LO + 1],
            lhsT=ones19[:],
            rhs=uf_sb[:, 32:33],
            start=False,
            stop=False,
        )

    state = {"banks_done": 0}
    qcols_sb = sb.tile([128, NT], F16)

    def emit_front(t0, ch):
        """doc DMA + matvecs: emitted a chunk ahead so the PE queue never
        head-blocks the next chunk's matvecs behind this chunk's segmats."""
        doc_t = docpool.tile([128, CHMAX * 128], F16, tag="doc")
        nc.sync.dma_start(
            out=doc_t[:, : ch * 128],
            in_=docp[:, NT + t0 * 128 : NT + (t0 + ch) * 128],
        )
        if t0 == 0:
            nc.scalar.dma_start(out=qcols_sb[:], in_=docp[:, 0:NT])
        scores = scpool.tile([128, CHMAX], F32, tag="sc")
        for tt in range(ch):
            nc.tensor.matmul(
                out=scores[:, tt : tt + 1],
                lhsT=doc_t[:, tt * 128 : (tt + 1) * 128],
                rhs=qcols_sb[:, t0 + tt : t0 + tt + 1],
                start=True,
                stop=True,
            )
        return scores

    def emit_rest(t0, ch, scores):
        # attn = exp(s) straight from PSUM in bf16 (s <= ~83 < ln(bf16 max));
        # masking is free: hosts sets ids=-1 at invalid positions, so both
        # one-hots (incl. the is_ge Z column) are all-zero there
        attn = wkpool.tile([128, CHMAX], BF16, tag="attn")
        nc.scalar.activation(
            out=attn[:, :ch], in_=scores[:, :ch], func=AF.Exp, bias=zcol[:, 0:1], scale=1.0
        )
        oh_t = ohpool.tile([128, LO * CHMAX], BF16, tag="oh")
        oh_v = oh_t[:].rearrange("p (l t) -> p l t", t=CHMAX)
        nc.vector.tensor_tensor(
            out=oh_v[:, :, 0:ch],
            in0=idlo[:, t0 : t0 + ch]
            .rearrange("p (o t) -> p o t", o=1)
            .to_broadcast([128, LO, ch]),
            in1=iota33.rearrange("p (l t) -> p l t", t=CHMAX)[:, :, 0:ch],
            op=ALU.is_equal,
        )
        nc.vector.tensor_scalar(
            out=oh_t[:, 32 * CHMAX : 32 * CHMAX + ch],
            in0=idlo[:, t0 : t0 + ch],
            scalar1=0,
            scalar2=None,
            op0=ALU.is_ge,
        )
        w19 = ohpool.tile([128, HI * CHMAX], BF16, tag="w19")
        w19_v = w19[:].rearrange("p (h t) -> p h t", t=CHMAX)
        nc.vector.tensor_tensor(
            out=w19_v[:, :, 0:ch],
            in0=idhi[:, t0 : t0 + ch]
            .rearrange("p (o t) -> p o t", o=1)
            .to_broadcast([128, HI, ch]),
            in1=iota19.rearrange("p (h t) -> p h t", t=CHMAX)[:, :, 0:ch],
            op=ALU.is_equal,
        )
        w19a = ohpool.tile([128, HI * CHMAX], BF16, tag="w19a")
        w19a_v = w19a[:].rearrange("p (h t) -> p h t", t=CHMAX)
        nc.vector.tensor_tensor(
            out=w19a_v[:, :, 0:ch],
            in0=w19_v[:, :, 0:ch],
            in1=attn[:, :ch]
            .rearrange("p (o t) -> p o t", o=1)
            .to_broadcast([128, HI, ch]),
            op=ALU.mult,
        )
        for tt in range(ch):
            t = t0 + tt
            g = t // 2
            if g < Gb:
                bk = g // GPB
                if bk not in u_tiles:
                    u_tiles[bk] = upool.tile(
                        [HI, GPB * LO], F32, tag="u", name=f"u{bk}"
                    )
                o = u_tiles[bk][:, (g % GPB) * LO : (g % GPB) * LO + LO]
                st = t % 2 == 0
                sp = t % 2 == 1
            else:
                if "uF" not in u_tiles:
                    u_tiles["uF"] = upool.tile([HI, LO], F32, tag="u", name="uF")
                o = u_tiles["uF"][:]
                st = t == 2 * Gb
                sp = t == NT - 1
            nc.tensor.matmul(
                out=o, lhsT=w19a_v[:, :, tt], rhs=oh_v[:, :, tt], start=st, stop=sp
            )
        tend = t0 + ch
        while state["banks_done"] < NBb and 2 * min(
            (state["banks_done"] + 1) * GPB, Gb
        ) <= tend:
            b = state["banks_done"]
            done = 2 * min((b + 1) * GPB, Gb)
            # the LAST bank's chain is the tail's critical path: schedule its
            # copy immediately and its round-trip one step later
            d = 0 if b == NBb - 1 else 16
            pending_copy.append((done + d, b))
            state["banks_done"] += 1
        while pending_copy and pending_copy[0][0] <= tend:
            due, b = pending_copy.pop(0)
            pending_rt.append((due + 16, b, finalize_bank_copy(b)))
        while pending_rt and pending_rt[0][0] <= tend:
            due, b, usb_t = pending_rt.pop(0)
            pending_mms.append((due + 24, finalize_bank_rt(b, usb_t)))
        while pending_mms and pending_mms[0][0] <= tend:
            pending_mms.pop(0)[1]()

    # ---- main stream (software-pipelined emission) ----
    t0 = 0
    prev = None
    pending_copy = []
    pending_rt = []
    pending_mms = []
    for ch in sizes:
        scores = emit_front(t0, ch)
        if prev is not None:
            emit_rest(*prev)
        prev = (t0, ch, scores)
        t0 += ch
    emit_rest(*prev)
    assert state["banks_done"] == NBb, (state["banks_done"], NBb)
    # leftover bank round-trips (their group data is long since ready)
    while pending_copy:
        _, b = pending_copy.pop(0)
        pending_rt.append((0, b, finalize_bank_copy(b)))
    while pending_rt:
        _, b, usb_t = pending_rt.pop(0)
        pending_mms.append((0, finalize_bank_rt(b, usb_t)))
    if F > 0:
        finalize_fast()
    # leftover Sel matmuls go AFTER the fast-path matmuls: their xp read may
    # still be in flight and a parked matmul head-blocks the PE queue
    while pending_mms:
        pending_mms.pop(0)[1]()

    if NBb == 0:
        # no bank matmuls: close the accumulation groups with zero matmuls
        # (the uF identity matmuls only touch sub-regions of row 0)
        nc.tensor.matmul(
            out=A_ps[:], lhsT=zrow8[:], rhs=zwide[:, 0 : 13 * LO], start=False, stop=True
        )
        nc.tensor.matmul(
            out=B_ps[:], lhsT=zrow8[:], rhs=zwide[:, 0 : 6 * LO + 1], start=False, stop=True
        )

    # ---- finalize: invZ, then logits = Ln(u*invZ + eps) straight from PSUM ----
    zz = sb.tile([BL, 1], F32)
    nc.vector.reciprocal(out=zz[:], in_=B_ps[:, 6 * LO : 6 * LO + 1])
    lg = sb.tile([BL, OUTE], F32)
    nc.scalar.activation(
        out=lg[:, 0:416].rearrange("j (h l) -> j h l", h=13),
        in_=A_ps[:].rearrange("j (h l) -> j h l", h=13)[:, :, 0:32],
        func=AF.Ln,
        bias=epscol[:, 0:1],
        scale=zz[:, 0:1],
    )
    nc.scalar.activation(
        out=lg[:, 416:512].rearrange("j (h l) -> j h l", h=3),
        in_=B_ps[:, 0 : 6 * LO].rearrange("j (h l) -> j h l", h=6)[:, 0:3, 0:32],
        func=AF.Ln,
        bias=epscol[:, 0:1],
        scale=zz[:, 0:1],
    )
    nc.sync.dma_start(out=out, in_=lg[:])


def build_program(NT, F):
    Gn = NT // 2
    Gb = Gn - F
    NBb = (Gb + GPB - 1) // GPB
    nc = bacc.Bacc(
        "TRN2",
        target_bir_lowering=False,
        debug=False,
        enable_asserts=False,
        num_devices=1,
    )
    docp = nc.dram_tensor(
        "docp", [128, NT + NT * 128], F16, kind="ExternalInput"
    ).ap()  # cols [0:NT] = per-tile q values; [NT:] = packed docT
    ids_d = nc.dram_tensor("ids", [128, 2 * NT], I16, kind="ExternalInput").ap()
    selb_d = nc.dram_tensor(
        "selb", [GPB, max(NBb, 1) * BL], BF16, kind="ExternalInput"
    ).ap()
    eye_d = nc.dram_tensor("eye19", [HI, HI], BF16, kind="ExternalInput").ap()
    xdram = nc.dram_tensor(
        "xdram", [max(Gb, 1), HI * LO], BF16, kind="ExternalInput"
    ).ap()
    out = nc.dram_tensor("out", [BL, OUTE], F32, kind="ExternalOutput").ap()

    tensors = (out, docp, ids_d, selb_d, eye_d, xdram)
    with tile.TileContext(nc) as tc:
        with ExitStack() as ctx:
            emit_kernel(ctx, tc, NT, F, tensors)
    nc.compile()
    return nc


def make_in_maps(doc_emb, query_emb, doc_ids, seq_length, plan):
    NT = plan["NT"]
    F = plan["F"]
    Gn = NT // 2
    Gb = Gn - F
    NBb = (Gb + GPB - 1) // GPB
    gt = plan["gt"]
    L = plan["L"]
    eye = np.eye(HI, dtype=np.float32).astype(BF16NP)
    in_maps = []
    for c in range(NCORES):
        bs = plan["assign"][c]
        docq = np.zeros((128, NT + NT * 128), np.float16)
        docT = docq[:, NT:].reshape(128, NT, 128)
        qcols = docq[:, :NT]
        ids2 = np.full((128, 2 * NT), -1, np.int16)
        idlo = ids2[:, :NT]
        idhi = ids2[:, NT:]
        selb = np.zeros((GPB, max(NBb, 1) * BL), BF16NP)
        t0 = 0
        p = np.arange(128)
        for j, b in enumerate(bs):
            nt = int(gt[b])
            lj = int(L[b])
            npos = min(nt * 128, S)
            seg = np.zeros((nt * 128, E), np.float32)
            seg[:npos] = doc_emb[b, :npos, :]
            docT[:, t0 : t0 + nt, :] = (
                seg.reshape(nt, 128, E).transpose(2, 0, 1).astype(np.float16)
            )
            qcols[:, t0 : t0 + nt] = query_emb[b].astype(np.float16)[:, None]
            svals = (np.arange(nt) * 128)[None, :] + p[:, None]
            valid = svals < lj
            idseg = np.zeros(nt * 128, np.int32)
            idseg[:npos] = doc_ids[b, :npos]
            idseg = idseg.reshape(nt, 128).T
            idlo[:, t0 : t0 + nt] = np.where(valid, idseg & 31, -1).astype(np.int16)
            idhi[:, t0 : t0 + nt] = np.where(valid, idseg >> 5, -1).astype(np.int16)
            slot = (j + 1) % BL  # stream-last batch -> slot 0 (fast path)
            for g in range(t0 // 2, (t0 + nt) // 2):
                if g < Gb:
                    selb[g % GPB, (g // GPB) * BL + slot] = 1.0
                else:
                    # fast groups are added straight into slot 0
                    assert slot == 0, (c, j, g, Gb)
            t0 += nt
        in_maps.append(
            {
                "docp": docq,
                "ids": ids2,
                "selb": selb,
                "eye19": eye,
                "xdram": np.zeros((max(Gb, 1), HI * LO), BF16NP),
            }
        )
    return in_maps


_CACHE = {}


def _get_program(key=None):
    if key is None:
        key = _CACHE.get("last_key")
        assert key is not None, "no program built yet"
    if key not in _CACHE:
        _CACHE[key] = build_program(*key)
    _CACHE["last_key"] = key
    return _CACHE[key]


def kernel(**inputs):
    doc_emb = np.asarray(inputs["doc_emb"], dtype=np.float32)
    query_emb = np.asarray(inputs["query_emb"], dtype=np.float32)
    doc_ids = np.asarray(inputs["doc_ids"], dtype=np.int32)
    seq_length = np.asarray(inputs["seq_length"], dtype=np.int32)

    plan = make_plan(seq_length)
    nc = _get_program((plan["NT"], plan["F"]))
    in_maps = make_in_maps(doc_emb, query_emb, doc_ids, seq_length, plan)
    res = bass_utils.run_bass_kernel_spmd(nc, in_maps, core_ids=list(range(NCORES)))
    out = np.zeros((B, OUTE), np.float32)
    for c in range(NCORES):
        o = np.asarray(res.results[c]["out"], dtype=np.float32)
        for j, b in enumerate(plan["assign"][c]):
            out[b] = o[(j + 1) % BL]
    return out
